# revision 16
# baseline (speedup 1.0000x reference)
"""Trainium2 Bass kernel for the LogSoftmax dual-stream attention module.

Math (per batch b, head h):
    qkv = x @ w_qkv ; q,k,v = split(qkv); q2 = qoir
    dots  = scale * q  @ k^T ; dots2 = scale * q2 @ k^T
    attn  = log_softmax(dots) = scale*dots_raw - lse       (log-probs!)
    out   = attn @ v  = scale * q @ (k^T v) - lse  (x) colsum(v)
    qout  = attn2 @ v = scale * q2 @ (k^T v) - lse2 (x) colsum(v)
    x_new = merge(out) @ w_out + x ; q_new = merge(qout) + qoir

The factorization removes the O(N^2) attn@V matmul entirely; the only O(N^2)
work is lse = ln(rowsum(exp(dots))), computed on ScalarE with the fused
activation accum_out (exp + row-sum in one instruction), dots on TensorE.

Sharding: 8 cores = (batch 0..3) x (row-half 0..1). Each core gets the full
2048 keys of its batch (rows permuted so its own 1024 query rows come first —
all key-side reductions are permutation invariant), computes its 1024 rows of
both outputs. No collectives.
"""

import numpy as np

B, N, DIM = 4, 2048, 512
HEADS, DH = 8, 64
INNER = HEADS * DH          # 512
ROWS = N // 2               # 1024 query rows per core
SCALE = DH ** -0.5          # 0.125
NCORES = 8

P = 128                     # partitions
NJT = N // P                # 16 key j-tiles
NIT = ROWS // P             # 8 query i-tiles


def build_bass():
    import concourse.bass as bass
    import concourse.mybir as mybir
    import concourse.tile as tile
    from concourse import bacc
    from concourse.masks import make_identity

    f32 = mybir.dt.float32
    bf16 = mybir.dt.bfloat16
    AF = mybir.ActivationFunctionType

    nc = bacc.Bacc()

    x_b = nc.declare_dram_parameter("x_b", [N, DIM], f32, isOutput=False)
    qoir_r = nc.declare_dram_parameter("qoir_r", [ROWS, INNER], f32, isOutput=False)
    w_qkv = nc.declare_dram_parameter("w_qkv", [DIM, 3 * INNER], f32, isOutput=False)
    w_out = nc.declare_dram_parameter("w_out", [INNER, DIM], f32, isOutput=False)
    maskB_in = nc.declare_dram_parameter("maskB_in", [8, INNER], f32, isOutput=False)
    xnew = nc.declare_dram_parameter("xnew_p", [ROWS, DIM], f32, isOutput=True)
    qnew = nc.declare_dram_parameter("qnew_p", [ROWS, INNER], f32, isOutput=True)

    with tile.TileContext(nc) as tc:
        with (
            tc.tile_pool(name="sb", bufs=1) as sb,
            tc.tile_pool(name="ps", bufs=2, space="PSUM") as ps,
        ):
            # ---------------- persistent SBUF ----------------
            wq = [sb.tile([P, 3 * INNER], f32, name=f"wq{d}", tag=f"wq{d}") for d in range(4)]
            wqb = [sb.tile([P, 3 * INNER], bf16, name=f"wqb{d}", tag=f"wqb{d}") for d in range(4)]
            wo = [sb.tile([P, DIM], f32, name=f"wo{d}", tag=f"wo{d}") for d in range(4)]
            xn = [sb.tile([P, DIM], f32, name=f"xn{j}", tag=f"xn{j}") for j in range(NJT)]
            xnb = [sb.tile([P, DIM], bf16, name=f"xnb{j}", tag=f"xnb{j}") for j in range(NJT)]
            q2n = [sb.tile([P, INNER], f32, name=f"q2n{j}", tag=f"q2n{j}") for j in range(NIT)]
            q2nb = [sb.tile([P, INNER], bf16, name=f"q2nb{j}", tag=f"q2nb{j}") for j in range(NIT)]
            xT = [sb.tile([P, N], bf16, name=f"xT{d}", tag=f"xT{d}") for d in range(4)]
            q2T = [sb.tile([P, ROWS], bf16, name=f"q2T{d}", tag=f"q2T{d}") for d in range(4)]
            QT = [sb.tile([P, ROWS], bf16, name=f"QT{t}", tag=f"QT{t}") for t in range(4)]
            KT = [sb.tile([P, N], bf16, name=f"KT{t}", tag=f"KT{t}") for t in range(4)]
            ktvT_acc = sb.tile([P, 2 * P], f32, name="ktvT_acc")
            colv_acc = sb.tile([1, INNER], f32, name="colv_acc")
            sumexp_all = sb.tile([P, P], f32, name="sumexp_all")
            lse_all = sb.tile([P, P], f32, name="lse_all")
            identf = sb.tile([P, P], f32, name="identf")
            identb = sb.tile([P, P], bf16, name="identb")
            ones_col = sb.tile([P, 1], bf16, name="ones_col")
            ones8 = sb.tile([1, 8], f32, name="ones8")
            maskA = [sb.tile([P, 8], f32, name=f"maskA{t}", tag=f"maskA{t}") for t in range(4)]
            mkA = [sb.tile([P, 8], f32, name=f"mkA{t}", tag=f"mkA{t}") for t in range(4)]
            maskB = sb.tile([8, INNER], f32, name="maskB")
            colvT_sb = sb.tile([P, 4], f32, name="colvT_sb")
            ktvT_sb = sb.tile([P, 2 * P], f32, name="ktvT_sb")
            bd = [sb.tile([P, P], f32, name=f"bd{t}", tag=f"bd{t}") for t in range(4)]
            At_sb = [sb.tile([P, DIM], bf16, name=f"At{t}", tag=f"At{t}") for t in range(4)]
            B_sb = [sb.tile([P, P], bf16, name=f"Bt{t}", tag=f"Bt{t}") for t in range(4)]
            CCx = sb.tile([8, DIM], f32, name="CCx")
            CCq = sb.tile([8, INNER], f32, name="CCq")

            # ---------------- constants ----------------
            make_identity(nc, identf)
            make_identity(nc, identb)
            nc.gpsimd.memset(ones_col, 1.0)  # bf16: 1.0 exact
            nc.gpsimd.memset(ones8, 1.0)
            for t in range(4):
                nc.gpsimd.memset(maskA[t], 0.0)
                # inner row r of tile t belongs to head 2t + r//64; mask col = head
                nc.gpsimd.memset(maskA[t][0:64, 2 * t : 2 * t + 1], 1.0)
                nc.gpsimd.memset(maskA[t][64:P, 2 * t + 1 : 2 * t + 2], 1.0)
            nc.sync.dma_start(maskB, maskB_in[:, :])

            # ---------------- DMA in ----------------
            for j in range(NJT):
                nc.sync.dma_start(xn[j], x_b[P * j : P * (j + 1), :])
            for d in range(4):
                nc.sync.dma_start(wq[d], w_qkv[P * d : P * (d + 1), :])
                nc.sync.dma_start(wo[d], w_out[P * d : P * (d + 1), :])
            for j in range(NIT):
                nc.sync.dma_start(q2n[j], qoir_r[P * j : P * (j + 1), :])

            # ---------------- bf16 casts ----------------
            for j in range(NJT):
                nc.vector.tensor_copy(xnb[j], xn[j])
            for j in range(NIT):
                nc.vector.tensor_copy(q2nb[j], q2n[j])
            for d in range(4):
                nc.vector.tensor_copy(wqb[d], wq[d])

            # ---------------- transposes: x -> xT via DMA xbar (PE stays free) ----------------
            for j in range(NJT):
                for d in range(4):
                    nc.sync.dma_start(
                        xT[d][:, P * j : P * (j + 1)],
                        xnb[j][:, P * d : P * (d + 1)],
                        transpose=True,
                    )

            # ---------------- transposes: qoir -> q2T via DMA xbar ----------------
            for j in range(NIT):
                for d in range(4):
                    nc.sync.dma_start(
                        q2T[d][:, P * j : P * (j + 1)],
                        q2nb[j][:, P * d : P * (d + 1)],
                        transpose=True,
                    )

            # ---------------- helpers ----------------
            def project_KT(t):
                for jc in range(4):
                    kp = ps.tile([P, DIM], f32, tag="big", name=f"kp{t}{jc}")
                    for d in range(4):
                        nc.tensor.matmul(
                            kp,
                            wqb[d][:, INNER + P * t : INNER + P * (t + 1)],
                            xT[d][:, DIM * jc : DIM * (jc + 1)],
                            start=(d == 0),
                            stop=(d == 3),
                        )
                    nc.vector.tensor_copy(KT[t][:, DIM * jc : DIM * (jc + 1)], kp)

            def project_QT(t):
                for ic in range(2):
                    qp = ps.tile([P, DIM], f32, tag="big", name=f"qp{t}{ic}")
                    for d in range(4):
                        nc.tensor.matmul(
                            qp,
                            wqb[d][:, P * t : P * (t + 1)],
                            xT[d][:, DIM * ic : DIM * (ic + 1)],
                            start=(d == 0),
                            stop=(d == 3),
                        )
                    nc.vector.tensor_copy(QT[t][:, DIM * ic : DIM * (ic + 1)], qp)

            def dots_pair(it, s, h):
                col = 16 * it + 8 * s + h
                dp = ps.tile([P, N], f32, tag="big", name=f"dp{col}")
                src = QT if s == 0 else q2T
                r0 = (h % 2) * DH
                lhsT = src[h // 2][r0 : r0 + DH, P * it : P * (it + 1)]
                for jc in range(4):
                    nc.tensor.matmul(
                        dp[:, DIM * jc : DIM * (jc + 1)],
                        lhsT,
                        KT[h // 2][r0 : r0 + DH, DIM * jc : DIM * (jc + 1)],
                        start=True,
                        stop=True,
                    )
                escr = sb.tile([P, N], bf16, tag="escr", bufs=2, name=f"escr{col}")
                nc.scalar.activation(
                    escr,
                    dp,
                    AF.Exp,
                    scale=SCALE,
                    accum_out=sumexp_all[:, col : col + 1],
                )

            def kv_work(j16):
                vnp = ps.tile([P, DIM], f32, tag="big", name=f"vnp{j16}")
                for d in range(4):
                    nc.tensor.matmul(
                        vnp,
                        xT[d][:, P * j16 : P * (j16 + 1)],
                        wqb[d][:, 2 * INNER : 3 * INNER],
                        start=(d == 0),
                        stop=(d == 3),
                    )
                kn_sb = sb.tile([P, DIM], bf16, tag="kn", bufs=2, name=f"kn{j16}")
                vn_sb = sb.tile([P, DIM], bf16, tag="vn", bufs=2, name=f"vn{j16}")
                for t in range(4):
                    nc.sync.dma_start(
                        kn_sb[:, P * t : P * (t + 1)],
                        KT[t][:, P * j16 : P * (j16 + 1)],
                        transpose=True,
                    )
                nc.vector.tensor_copy(vn_sb, vnp)
                # ktv^T partial: head h -> rows (h%2)*64, cols 64*(h//2); colv into bank 1
                kvp = ps.tile([P, 2 * DIM], f32, tag="big", name=f"kvp{j16}")
                for h in range(HEADS):
                    nc.tensor.matmul(
                        kvp[(h % 2) * DH : (h % 2 + 1) * DH, DH * (h // 2) : DH * (h // 2 + 1)],
                        vn_sb[:, DH * h : DH * (h + 1)],
                        kn_sb[:, DH * h : DH * (h + 1)],
                        start=True,
                        stop=True,
                    )
                nc.tensor.matmul(kvp[0:1, DIM : DIM + INNER], ones_col, vn_sb, start=True, stop=True)
                if j16 == 0:
                    nc.vector.tensor_copy(ktvT_acc, kvp[:, 0 : 2 * P])
                    nc.vector.tensor_copy(colv_acc, kvp[0:1, DIM : DIM + INNER])
                else:
                    nc.vector.tensor_add(ktvT_acc, ktvT_acc, kvp[:, 0 : 2 * P])
                    nc.vector.tensor_add(colv_acc, colv_acc, kvp[0:1, DIM : DIM + INNER])

            # ---------------- main loop: project per head-pair, dots, kv interleave ----------------
            def finalize_ktv():
                pass  # placeholder, replaced below

            for t in range(4):
                if t == 0:
                    project_KT(0)
                project_QT(t)
                for it in range(NIT):
                    for s in range(2):
                        for h in (2 * t, 2 * t + 1):
                            dots_pair(it, s, h)
                    if t == 0 and it == 3:
                        project_KT(1)
                        project_KT(2)
                        project_KT(3)
                    if t in (1, 2):
                        kv_work(8 * (t - 1) + it)

            # ---------------- finalize ktv / colv -> Â, B, CCx, CCq ----------------
            nc.vector.tensor_scalar_mul(ktvT_sb, ktvT_acc, SCALE)
            for t in range(4):
                nc.gpsimd.memset(bd[t], 0.0)
                nc.vector.tensor_copy(bd[t][0:DH, 0:DH], ktvT_sb[0:DH, DH * t : DH * (t + 1)])
                nc.vector.tensor_copy(
                    bd[t][DH:P, DH:P], ktvT_sb[DH:P, DH * t : DH * (t + 1)]
                )
            for t in range(4):
                ap_ = ps.tile([P, DIM], f32, tag="big", name=f"ap{t}")
                nc.tensor.matmul(ap_, bd[t], wo[t], start=True, stop=True)
                nc.vector.tensor_copy(At_sb[t], ap_)
            for t in range(4):
                bp = ps.tile([P, P], f32, tag="big", name=f"bp{t}")
                nc.tensor.transpose(bp, bd[t], identf)
                nc.vector.tensor_copy(B_sb[t], bp)
            # colv^T columns via K=1 matmuls against 1.0
            cvt = ps.tile([P, 4], f32, tag="big", name="cvt")
            for t in range(4):
                nc.tensor.matmul(
                    cvt[:, t : t + 1],
                    colv_acc[:, P * t : P * (t + 1)],
                    identf[0:1, 0:1],
                    start=True,
                    stop=True,
                )
            nc.vector.tensor_copy(colvT_sb, cvt)
            for t in range(4):
                nc.vector.tensor_scalar_mul(mkA[t], maskA[t], colvT_sb[:, t : t + 1])
            cp = ps.tile([8, DIM], f32, tag="big", name="cp")
            for t in range(4):
                nc.tensor.matmul(cp, mkA[t], wo[t], start=(t == 0), stop=(t == 3))
            nc.vector.tensor_scalar_mul(CCx, cp, -1.0)
            bc = ps.tile([8, INNER], f32, tag="big", name="bc")
            nc.tensor.matmul(bc, ones8, colv_acc, start=True, stop=True)
            nc.vector.tensor_mul(CCq, bc, maskB)

            # ---------------- lse + assembly ----------------
            nc.scalar.activation(lse_all, sumexp_all, AF.Ln)
            for it in range(NIT):
                ltx = ps.tile([8, P], f32, tag="big", name=f"ltx{it}")
                nc.tensor.transpose(ltx, lse_all[:, 16 * it : 16 * it + 8], identf)
                ltq = ps.tile([8, P], f32, tag="big", name=f"ltq{it}")
                nc.tensor.transpose(ltq, lse_all[:, 16 * it + 8 : 16 * it + 16], identf)
                lx_sb = sb.tile([8, P], f32, tag="lx", bufs=2, name=f"lx{it}")
                lq_sb = sb.tile([8, P], f32, tag="lq", bufs=2, name=f"lq{it}")
                nc.vector.tensor_copy(lx_sb, ltx)
                nc.vector.tensor_copy(lq_sb, ltq)

                xp = ps.tile([P, DIM], f32, tag="big", name=f"xp{it}")
                for t in range(4):
                    nc.tensor.matmul(
                        xp, QT[t][:, P * it : P * (it + 1)], At_sb[t],
                        start=(t == 0), stop=False,
                    )
                nc.tensor.matmul(xp, lx_sb, CCx, start=False, stop=True)
                xst = sb.tile([P, DIM], f32, tag="xst", bufs=2, name=f"xst{it}")
                nc.vector.tensor_add(xst, xp, xn[it])
                nc.sync.dma_start(xnew[P * it : P * (it + 1), :], xst)

                qp = ps.tile([P, INNER], f32, tag="big", name=f"qpo{it}")
                for t in range(4):
                    reg = qp[:, P * t : P * (t + 1)]
                    nc.tensor.matmul(
                        reg, lq_sb, CCq[:, P * t : P * (t + 1)], start=True, stop=False
                    )
                    nc.tensor.matmul(
                        reg,
                        q2T[t][:, P * it : P * (it + 1)],
                        B_sb[t],
                        start=False, stop=True,
                    )
                qst = sb.tile([P, INNER], f32, tag="qst", bufs=2, name=f"qst{it}")
                nc.vector.tensor_add(qst, qp, q2n[it])
                nc.sync.dma_start(qnew[P * it : P * (it + 1), :], qst)

    nc.compile()
    return nc


_CACHE = {}


def _get_nc():
    if "nc" not in _CACHE:
        _CACHE["nc"] = build_bass()
    return _CACHE["nc"]


def _shard_inputs(x, qoir):
    """Per-core input maps. Core c: batch c//2, row-half c%2, own rows first."""
    in_maps = []
    for c in range(NCORES):
        b, half = c // 2, c % 2
        mine = x[b, half * ROWS : (half + 1) * ROWS]
        other = x[b, (1 - half) * ROWS : (2 - half) * ROWS]
        in_maps.append(
            {
                "x_b": np.ascontiguousarray(np.concatenate([mine, other], axis=0)),
                "qoir_r": np.ascontiguousarray(qoir[b, half * ROWS : (half + 1) * ROWS]),
            }
        )
    return in_maps


def _maskB():
    mb = np.zeros((8, INNER), dtype=np.float32)
    for h in range(8):
        mb[h, DH * h : DH * (h + 1)] = -1.0
    return mb


def kernel(x, qoir, w_qkv, w_out):
    from concourse.bass_utils import run_bass_kernel_spmd

    x = np.asarray(x, dtype=np.float32)
    qoir = np.asarray(qoir, dtype=np.float32)
    w_qkv = np.ascontiguousarray(np.asarray(w_qkv, dtype=np.float32))
    w_out = np.ascontiguousarray(np.asarray(w_out, dtype=np.float32))

    nc = _get_nc()
    in_maps = _shard_inputs(x, qoir)
    for m in in_maps:
        m["w_qkv"] = w_qkv
        m["w_out"] = w_out
        m["maskB_in"] = _maskB()

    res = run_bass_kernel_spmd(nc, in_maps, core_ids=list(range(NCORES)))
    x_new = np.empty((B, N, DIM), dtype=np.float32)
    q_new = np.empty((B, N, INNER), dtype=np.float32)
    for c in range(NCORES):
        b, half = c // 2, c % 2
        rows = slice(half * ROWS, (half + 1) * ROWS)
        x_new[b, rows] = res.results[c]["xnew_p"]
        q_new[b, rows] = res.results[c]["qnew_p"]
    return (x_new, q_new)


# revision 18
# speedup vs baseline: 1.0425x; 1.0425x over previous
"""Trainium2 Bass kernel for the LogSoftmax dual-stream attention module.

Math (per batch b, head h):
    qkv = x @ w_qkv ; q,k,v = split(qkv); q2 = qoir
    dots  = scale * q  @ k^T ; dots2 = scale * q2 @ k^T
    attn  = log_softmax(dots) = scale*dots_raw - lse       (log-probs!)
    out   = attn @ v  = scale * q @ (k^T v) - lse  (x) colsum(v)
    qout  = attn2 @ v = scale * q2 @ (k^T v) - lse2 (x) colsum(v)
    x_new = merge(out) @ w_out + x ; q_new = merge(qout) + qoir

The factorization removes the O(N^2) attn@V matmul entirely; the only O(N^2)
work is lse = ln(rowsum(exp(dots))), computed on ScalarE with the fused
activation accum_out (exp + row-sum in one instruction), dots on TensorE.

Sharding: 8 cores = (batch 0..3) x (row-half 0..1). Each core gets the full
2048 keys of its batch (rows permuted so its own 1024 query rows come first —
all key-side reductions are permutation invariant), computes its 1024 rows of
both outputs. No collectives.
"""

import numpy as np

B, N, DIM = 4, 2048, 512
HEADS, DH = 8, 64
INNER = HEADS * DH          # 512
ROWS = N // 2               # 1024 query rows per core
SCALE = DH ** -0.5          # 0.125
NCORES = 8

P = 128                     # partitions
NJT = N // P                # 16 key j-tiles
NIT = ROWS // P             # 8 query i-tiles


def build_bass():
    import concourse.bass as bass
    import concourse.mybir as mybir
    import concourse.tile as tile
    from concourse import bacc
    from concourse.masks import make_identity

    f32 = mybir.dt.float32
    bf16 = mybir.dt.bfloat16
    AF = mybir.ActivationFunctionType

    nc = bacc.Bacc()

    x_b = nc.declare_dram_parameter("x_b", [N, DIM], f32, isOutput=False)
    qoir_r = nc.declare_dram_parameter("qoir_r", [ROWS, INNER], f32, isOutput=False)
    w_qkv = nc.declare_dram_parameter("w_qkv", [DIM, 3 * INNER], f32, isOutput=False)
    w_out = nc.declare_dram_parameter("w_out", [INNER, DIM], f32, isOutput=False)
    maskB_in = nc.declare_dram_parameter("maskB_in", [8, INNER], f32, isOutput=False)
    xnew = nc.declare_dram_parameter("xnew_p", [ROWS, DIM], f32, isOutput=True)
    qnew = nc.declare_dram_parameter("qnew_p", [ROWS, INNER], f32, isOutput=True)

    with tile.TileContext(nc) as tc:
        with (
            tc.tile_pool(name="sb", bufs=1) as sb,
            tc.tile_pool(name="ps", bufs=2, space="PSUM") as ps,
        ):
            # ---------------- persistent SBUF ----------------
            wq = [sb.tile([P, 3 * INNER], f32, name=f"wq{d}", tag=f"wq{d}") for d in range(4)]
            wqb = [sb.tile([P, 3 * INNER], bf16, name=f"wqb{d}", tag=f"wqb{d}") for d in range(4)]
            wo = [sb.tile([P, DIM], f32, name=f"wo{d}", tag=f"wo{d}") for d in range(4)]
            xn = [
                sb.tile([P, DIM], f32, name=f"xn{j}", tag=f"xn{j}")
                if j < NIT
                else sb.tile([P, DIM], f32, name=f"xn{j}", tag="xnrot", bufs=3)
                for j in range(NJT)
            ]
            xnb = [sb.tile([P, DIM], bf16, name=f"xnb{j}", tag=f"xnb{j}") for j in range(NJT)]
            q2n = [sb.tile([P, INNER], f32, name=f"q2n{j}", tag=f"q2n{j}") for j in range(NIT)]
            q2nb = [sb.tile([P, INNER], bf16, name=f"q2nb{j}", tag=f"q2nb{j}") for j in range(NIT)]
            xT = [sb.tile([P, N], bf16, name=f"xT{d}", tag=f"xT{d}") for d in range(4)]
            q2T = [sb.tile([P, ROWS], bf16, name=f"q2T{d}", tag=f"q2T{d}") for d in range(4)]
            QT = [sb.tile([P, ROWS], bf16, name=f"QT{t}", tag=f"QT{t}") for t in range(4)]
            KT = [sb.tile([P, N], bf16, name=f"KT{t}", tag=f"KT{t}") for t in range(4)]
            ktvT_acc = sb.tile([P, 2 * P], f32, name="ktvT_acc")
            colv_acc = sb.tile([1, INNER], f32, name="colv_acc")
            sumexp_all = sb.tile([P, P], f32, name="sumexp_all")
            lse_all = sb.tile([P, P], f32, name="lse_all")
            identf = sb.tile([P, P], f32, name="identf")
            identb = sb.tile([P, P], bf16, name="identb")
            ones_col = sb.tile([P, 1], bf16, name="ones_col")
            ones8 = sb.tile([1, 8], f32, name="ones8")
            maskA = [sb.tile([P, 8], f32, name=f"maskA{t}", tag=f"maskA{t}") for t in range(4)]
            mkA = [sb.tile([P, 8], f32, name=f"mkA{t}", tag=f"mkA{t}") for t in range(4)]
            maskB = sb.tile([8, INNER], f32, name="maskB")
            colvT_sb = sb.tile([P, 4], f32, name="colvT_sb")
            ktvT_sb = sb.tile([P, 2 * P], f32, name="ktvT_sb")
            bd = [sb.tile([P, P], f32, name=f"bd{t}", tag=f"bd{t}") for t in range(4)]
            At_sb = [sb.tile([P, DIM], bf16, name=f"At{t}", tag=f"At{t}") for t in range(4)]
            B_sb = [sb.tile([P, P], bf16, name=f"Bt{t}", tag=f"Bt{t}") for t in range(4)]
            CCx = sb.tile([8, DIM], f32, name="CCx")
            CCq = sb.tile([8, INNER], f32, name="CCq")

            # ---------------- constants ----------------
            make_identity(nc, identf)
            make_identity(nc, identb)
            nc.gpsimd.memset(ones_col, 1.0)  # bf16: 1.0 exact
            nc.gpsimd.memset(ones8, 1.0)
            for t in range(4):
                nc.gpsimd.memset(maskA[t], 0.0)
                # inner row r of tile t belongs to head 2t + r//64; mask col = head
                nc.gpsimd.memset(maskA[t][0:64, 2 * t : 2 * t + 1], 1.0)
                nc.gpsimd.memset(maskA[t][64:P, 2 * t + 1 : 2 * t + 2], 1.0)
            nc.sync.dma_start(maskB, maskB_in[:, :])

            # ---------------- DMA in ----------------
            for j in range(NJT):
                nc.sync.dma_start(xn[j], x_b[P * j : P * (j + 1), :])
            for d in range(4):
                nc.sync.dma_start(wq[d], w_qkv[P * d : P * (d + 1), :])
                nc.sync.dma_start(wo[d], w_out[P * d : P * (d + 1), :])
            for j in range(NIT):
                nc.sync.dma_start(q2n[j], qoir_r[P * j : P * (j + 1), :])

            # ---------------- bf16 casts ----------------
            for j in range(NJT):
                nc.vector.tensor_copy(xnb[j], xn[j])
            for j in range(NIT):
                nc.vector.tensor_copy(q2nb[j], q2n[j])
            for d in range(4):
                nc.vector.tensor_copy(wqb[d], wq[d])

            # ---------------- transposes: x -> xT (PE, packed psum groups) ----------------
            for g in range(4):
                ptr = ps.tile([P, 4 * DIM], bf16, tag="big", name=f"ptr{g}")
                for k in range(4):
                    for d in range(4):
                        nc.tensor.transpose(
                            ptr[:, DIM * k + P * d : DIM * k + P * (d + 1)],
                            xnb[4 * g + k][:, P * d : P * (d + 1)],
                            identb,
                        )
                for d in range(4):
                    src = ptr.rearrange("p (k x) -> p k x", x=DIM)[:, :, P * d : P * (d + 1)]
                    dst = xT[d][:, DIM * g : DIM * (g + 1)].rearrange("p (k x) -> p k x", x=P)
                    nc.vector.tensor_copy(dst, src)

            # ---------------- transposes: qoir -> q2T (PE) ----------------
            for g in range(2):
                ptr = ps.tile([P, 4 * INNER], bf16, tag="big", name=f"ptq{g}")
                for k in range(4):
                    for d in range(4):
                        nc.tensor.transpose(
                            ptr[:, INNER * k + P * d : INNER * k + P * (d + 1)],
                            q2nb[4 * g + k][:, P * d : P * (d + 1)],
                            identb,
                        )
                for d in range(4):
                    src = ptr.rearrange("p (k x) -> p k x", x=INNER)[:, :, P * d : P * (d + 1)]
                    dst = q2T[d][:, DIM * g : DIM * (g + 1)].rearrange("p (k x) -> p k x", x=P)
                    nc.vector.tensor_copy(dst, src)

            # ---------------- helpers ----------------
            from contextlib import contextmanager

            @contextmanager
            def backfill():
                save = tc.cur_priority
                tc.cur_priority = save + 1_000_000
                try:
                    yield
                finally:
                    tc.cur_priority = save

            def project_KT(t):
                for jc in range(4):
                    kp = ps.tile([P, DIM], f32, tag="big", name=f"kp{t}{jc}")
                    for d in range(4):
                        nc.tensor.matmul(
                            kp,
                            wqb[d][:, INNER + P * t : INNER + P * (t + 1)],
                            xT[d][:, DIM * jc : DIM * (jc + 1)],
                            start=(d == 0),
                            stop=(d == 3),
                        )
                    nc.vector.tensor_copy(KT[t][:, DIM * jc : DIM * (jc + 1)], kp)

            def project_QT(t):
                for ic in range(2):
                    qp = ps.tile([P, DIM], f32, tag="big", name=f"qp{t}{ic}")
                    for d in range(4):
                        nc.tensor.matmul(
                            qp,
                            wqb[d][:, P * t : P * (t + 1)],
                            xT[d][:, DIM * ic : DIM * (ic + 1)],
                            start=(d == 0),
                            stop=(d == 3),
                        )
                    nc.vector.tensor_copy(QT[t][:, DIM * ic : DIM * (ic + 1)], qp)

            def dots_pair(it, s, h):
                col = 16 * it + 8 * s + h
                dp = ps.tile([P, N], f32, tag="big", name=f"dp{col}")
                src = QT if s == 0 else q2T
                r0 = (h % 2) * DH
                lhsT = src[h // 2][r0 : r0 + DH, P * it : P * (it + 1)]
                for jc in range(4):
                    nc.tensor.matmul(
                        dp[:, DIM * jc : DIM * (jc + 1)],
                        lhsT,
                        KT[h // 2][r0 : r0 + DH, DIM * jc : DIM * (jc + 1)],
                        start=True,
                        stop=True,
                    )
                escr = sb.tile([P, N], bf16, tag="escr", bufs=3, name=f"escr{col}")
                nc.scalar.activation(
                    escr,
                    dp,
                    AF.Exp,
                    scale=SCALE,
                    accum_out=sumexp_all[:, col : col + 1],
                )

            def kv_work(j16):
                knp = ps.tile([P, DIM], f32, tag="big", name=f"knp{j16}")
                vnp = ps.tile([P, DIM], f32, tag="big", name=f"vnp{j16}")
                for d in range(4):
                    nc.tensor.matmul(
                        knp,
                        xT[d][:, P * j16 : P * (j16 + 1)],
                        wqb[d][:, INNER : 2 * INNER],
                        start=(d == 0),
                        stop=(d == 3),
                    )
                for d in range(4):
                    nc.tensor.matmul(
                        vnp,
                        xT[d][:, P * j16 : P * (j16 + 1)],
                        wqb[d][:, 2 * INNER : 3 * INNER],
                        start=(d == 0),
                        stop=(d == 3),
                    )
                kn_sb = sb.tile([P, DIM], bf16, tag="kn", bufs=2, name=f"kn{j16}")
                vn_sb = sb.tile([P, DIM], bf16, tag="vn", bufs=2, name=f"vn{j16}")
                nc.vector.tensor_copy(kn_sb, knp)
                nc.vector.tensor_copy(vn_sb, vnp)
                # ktv^T partial: head h -> rows (h%2)*64, cols 64*(h//2); colv into bank 1
                kvp = ps.tile([P, 2 * DIM], f32, tag="big", name=f"kvp{j16}")
                for h in range(HEADS):
                    nc.tensor.matmul(
                        kvp[(h % 2) * DH : (h % 2 + 1) * DH, DH * (h // 2) : DH * (h // 2 + 1)],
                        vn_sb[:, DH * h : DH * (h + 1)],
                        kn_sb[:, DH * h : DH * (h + 1)],
                        start=True,
                        stop=True,
                    )
                nc.tensor.matmul(kvp[0:1, DIM : DIM + INNER], ones_col, vn_sb, start=True, stop=True)
                if j16 == 0:
                    nc.vector.tensor_copy(ktvT_acc, kvp[:, 0 : 2 * P])
                    nc.vector.tensor_copy(colv_acc, kvp[0:1, DIM : DIM + INNER])
                else:
                    nc.vector.tensor_add(ktvT_acc, ktvT_acc, kvp[:, 0 : 2 * P])
                    nc.vector.tensor_add(colv_acc, colv_acc, kvp[0:1, DIM : DIM + INNER])

            # ---------------- main loop: project per head-pair, dots, kv interleave ----------------
            def finalize_ktv():
                pass  # placeholder, replaced below

            for t in range(4):
                if t == 0:
                    project_KT(0)
                    project_QT(0)
                else:
                    with backfill():
                        project_QT(t)
                for it in range(NIT):
                    for s in range(2):
                        for h in (2 * t, 2 * t + 1):
                            dots_pair(it, s, h)
                    if t == 0 and it == 3:
                        with backfill():
                            project_KT(1)
                            project_KT(2)
                            project_KT(3)
                    if t in (1, 2):
                        with backfill():
                            kv_work(8 * (t - 1) + it)

            # ---------------- finalize ktv / colv -> Â, B, CCx, CCq ----------------
            nc.vector.tensor_scalar_mul(ktvT_sb, ktvT_acc, SCALE)
            for t in range(4):
                nc.gpsimd.memset(bd[t], 0.0)
                nc.vector.tensor_copy(bd[t][0:DH, 0:DH], ktvT_sb[0:DH, DH * t : DH * (t + 1)])
                nc.vector.tensor_copy(
                    bd[t][DH:P, DH:P], ktvT_sb[DH:P, DH * t : DH * (t + 1)]
                )
            for t in range(4):
                ap_ = ps.tile([P, DIM], f32, tag="big", name=f"ap{t}")
                nc.tensor.matmul(ap_, bd[t], wo[t], start=True, stop=True)
                nc.vector.tensor_copy(At_sb[t], ap_)
            for t in range(4):
                bp = ps.tile([P, P], f32, tag="big", name=f"bp{t}")
                nc.tensor.transpose(bp, bd[t], identf)
                nc.vector.tensor_copy(B_sb[t], bp)
            # colv^T columns via K=1 matmuls against 1.0
            cvt = ps.tile([P, 4], f32, tag="big", name="cvt")
            for t in range(4):
                nc.tensor.matmul(
                    cvt[:, t : t + 1],
                    colv_acc[:, P * t : P * (t + 1)],
                    identf[0:1, 0:1],
                    start=True,
                    stop=True,
                )
            nc.vector.tensor_copy(colvT_sb, cvt)
            for t in range(4):
                nc.vector.tensor_scalar_mul(mkA[t], maskA[t], colvT_sb[:, t : t + 1])
            cp = ps.tile([8, DIM], f32, tag="big", name="cp")
            for t in range(4):
                nc.tensor.matmul(cp, mkA[t], wo[t], start=(t == 0), stop=(t == 3))
            nc.vector.tensor_scalar_mul(CCx, cp, -1.0)
            bc = ps.tile([8, INNER], f32, tag="big", name="bc")
            nc.tensor.matmul(bc, ones8, colv_acc, start=True, stop=True)
            nc.vector.tensor_mul(CCq, bc, maskB)

            # ---------------- lse + assembly ----------------
            nc.scalar.activation(lse_all, sumexp_all, AF.Ln)
            for it in range(NIT):
                ltx = ps.tile([8, P], f32, tag="big", name=f"ltx{it}")
                nc.tensor.transpose(ltx, lse_all[:, 16 * it : 16 * it + 8], identf)
                ltq = ps.tile([8, P], f32, tag="big", name=f"ltq{it}")
                nc.tensor.transpose(ltq, lse_all[:, 16 * it + 8 : 16 * it + 16], identf)
                lx_sb = sb.tile([8, P], f32, tag="lx", bufs=2, name=f"lx{it}")
                lq_sb = sb.tile([8, P], f32, tag="lq", bufs=2, name=f"lq{it}")
                nc.vector.tensor_copy(lx_sb, ltx)
                nc.vector.tensor_copy(lq_sb, ltq)

                xp = ps.tile([P, DIM], f32, tag="big", name=f"xp{it}")
                for t in range(4):
                    nc.tensor.matmul(
                        xp, QT[t][:, P * it : P * (it + 1)], At_sb[t],
                        start=(t == 0), stop=False,
                    )
                nc.tensor.matmul(xp, lx_sb, CCx, start=False, stop=True)
                xst = sb.tile([P, DIM], f32, tag="xst", bufs=2, name=f"xst{it}")
                nc.vector.tensor_add(xst, xp, xn[it])
                nc.sync.dma_start(xnew[P * it : P * (it + 1), :], xst)

                qp = ps.tile([P, INNER], f32, tag="big", name=f"qpo{it}")
                for t in range(4):
                    reg = qp[:, P * t : P * (t + 1)]
                    nc.tensor.matmul(
                        reg, lq_sb, CCq[:, P * t : P * (t + 1)], start=True, stop=False
                    )
                    nc.tensor.matmul(
                        reg,
                        q2T[t][:, P * it : P * (it + 1)],
                        B_sb[t],
                        start=False, stop=True,
                    )
                qst = sb.tile([P, INNER], f32, tag="qst", bufs=2, name=f"qst{it}")
                nc.vector.tensor_add(qst, qp, q2n[it])
                nc.sync.dma_start(qnew[P * it : P * (it + 1), :], qst)

    nc.compile()
    return nc


_CACHE = {}


def _get_nc():
    if "nc" not in _CACHE:
        _CACHE["nc"] = build_bass()
    return _CACHE["nc"]


def _shard_inputs(x, qoir):
    """Per-core input maps. Core c: batch c//2, row-half c%2, own rows first."""
    in_maps = []
    for c in range(NCORES):
        b, half = c // 2, c % 2
        mine = x[b, half * ROWS : (half + 1) * ROWS]
        other = x[b, (1 - half) * ROWS : (2 - half) * ROWS]
        in_maps.append(
            {
                "x_b": np.ascontiguousarray(np.concatenate([mine, other], axis=0)),
                "qoir_r": np.ascontiguousarray(qoir[b, half * ROWS : (half + 1) * ROWS]),
            }
        )
    return in_maps


def _maskB():
    mb = np.zeros((8, INNER), dtype=np.float32)
    for h in range(8):
        mb[h, DH * h : DH * (h + 1)] = -1.0
    return mb


def kernel(x, qoir, w_qkv, w_out):
    from concourse.bass_utils import run_bass_kernel_spmd

    x = np.asarray(x, dtype=np.float32)
    qoir = np.asarray(qoir, dtype=np.float32)
    w_qkv = np.ascontiguousarray(np.asarray(w_qkv, dtype=np.float32))
    w_out = np.ascontiguousarray(np.asarray(w_out, dtype=np.float32))

    nc = _get_nc()
    in_maps = _shard_inputs(x, qoir)
    for m in in_maps:
        m["w_qkv"] = w_qkv
        m["w_out"] = w_out
        m["maskB_in"] = _maskB()

    res = run_bass_kernel_spmd(nc, in_maps, core_ids=list(range(NCORES)))
    x_new = np.empty((B, N, DIM), dtype=np.float32)
    q_new = np.empty((B, N, INNER), dtype=np.float32)
    for c in range(NCORES):
        b, half = c // 2, c % 2
        rows = slice(half * ROWS, (half + 1) * ROWS)
        x_new[b, rows] = res.results[c]["xnew_p"]
        q_new[b, rows] = res.results[c]["qnew_p"]
    return (x_new, q_new)


# revision 24
# speedup vs baseline: 1.3066x; 1.2533x over previous
"""Trainium2 Bass kernel for the LogSoftmax dual-stream attention module.

Math (per batch b, head h):
    qkv = x @ w_qkv ; q,k,v = split(qkv); q2 = qoir
    dots  = scale * q  @ k^T ; dots2 = scale * q2 @ k^T
    attn  = log_softmax(dots) = scale*dots_raw - lse       (log-probs!)
    out   = attn @ v  = scale * q @ (k^T v) - lse  (x) colsum(v)
    qout  = attn2 @ v = scale * q2 @ (k^T v) - lse2 (x) colsum(v)
    x_new = merge(out) @ w_out + x ; q_new = merge(qout) + qoir

The factorization removes the O(N^2) attn@V matmul entirely; the only O(N^2)
work is lse = ln(rowsum(exp(dots))), computed on ScalarE with the fused
activation accum_out (exp + row-sum in one instruction), dots on TensorE.

Sharding: 8 cores = (batch 0..3) x (row-half 0..1). Each core gets the full
2048 keys of its batch (rows permuted so its own 1024 query rows come first —
all key-side reductions are permutation invariant), computes its 1024 rows of
both outputs. No collectives.
"""

import numpy as np

B, N, DIM = 4, 2048, 512
HEADS, DH = 8, 64
INNER = HEADS * DH          # 512
ROWS = N // 2               # 1024 query rows per core
SCALE = DH ** -0.5          # 0.125
NCORES = 8

P = 128                     # partitions
NJT = N // P                # 16 key j-tiles
NIT = ROWS // P             # 8 query i-tiles


def build_bass():
    import concourse.bass as bass
    import concourse.mybir as mybir
    import concourse.tile as tile
    from concourse import bacc
    from concourse.masks import make_identity
    from contextlib import contextmanager

    f32 = mybir.dt.float32
    bf16 = mybir.dt.bfloat16
    AF = mybir.ActivationFunctionType

    nc = bacc.Bacc()

    x_b = nc.declare_dram_parameter("x_b", [N, DIM], f32, isOutput=False)
    qoir_r = nc.declare_dram_parameter("qoir_r", [ROWS, INNER], f32, isOutput=False)
    w_qkv = nc.declare_dram_parameter("w_qkv", [DIM, 3 * INNER], f32, isOutput=False)
    w_out = nc.declare_dram_parameter("w_out", [INNER, DIM], f32, isOutput=False)
    maskB_in = nc.declare_dram_parameter("maskB_in", [8, INNER], f32, isOutput=False)
    xnew = nc.declare_dram_parameter("xnew_p", [ROWS, DIM], f32, isOutput=True)
    qnew = nc.declare_dram_parameter("qnew_p", [ROWS, INNER], f32, isOutput=True)

    NA = 3 * DIM  # 1536: EXP part A width

    with tile.TileContext(nc) as tc:
        with (
            tc.tile_pool(name="sb", bufs=1) as sb,
            tc.tile_pool(name="ps", bufs=2, space="PSUM") as ps,
        ):
            # ---------------- persistent SBUF ----------------
            wq = [sb.tile([P, 3 * INNER], f32, name=f"wq{d}", tag=f"wq{d}") for d in range(4)]
            wqb = [sb.tile([P, 3 * INNER], bf16, name=f"wqb{d}", tag=f"wqb{d}") for d in range(4)]
            wo = [sb.tile([P, DIM], f32, name=f"wo{d}", tag=f"wo{d}") for d in range(4)]
            xn = [
                sb.tile([P, DIM], f32, name=f"xn{j}", tag=f"xn{j}")
                if j < NIT
                else sb.tile([P, DIM], f32, name=f"xn{j}", tag="xnrot", bufs=3)
                for j in range(NJT)
            ]
            xnb = [sb.tile([P, DIM], bf16, name=f"xnb{j}", tag=f"xnb{j}") for j in range(NJT)]
            q2n = [sb.tile([P, INNER], f32, name=f"q2n{j}", tag=f"q2n{j}") for j in range(NIT)]
            q2nb = [sb.tile([P, INNER], bf16, name=f"q2nb{j}", tag=f"q2nb{j}") for j in range(NIT)]
            xT = [sb.tile([P, N], bf16, name=f"xT{d}", tag=f"xT{d}") for d in range(4)]
            q2T = [sb.tile([P, ROWS], bf16, name=f"q2T{d}", tag=f"q2T{d}") for d in range(4)]
            QT = [sb.tile([P, ROWS], bf16, name=f"QT{t}", tag=f"QT{t}") for t in range(4)]
            KT = [sb.tile([P, N], bf16, name=f"KT{t}", tag=f"KT{t}") for t in range(4)]
            ktvT_acc = sb.tile([P, 2 * P], f32, name="ktvT_acc")
            colv_acc = sb.tile([1, INNER], f32, name="colv_acc")
            pa_all = sb.tile([P, P], f32, name="pa_all")
            pb_all = sb.tile([P, P], f32, name="pb_all")
            se_all = sb.tile([P, P], f32, name="se_all")
            lse_all = sb.tile([P, P], f32, name="lse_all")
            identf = sb.tile([P, P], f32, name="identf")
            identb = sb.tile([P, P], bf16, name="identb")
            ones_col = sb.tile([P, 1], bf16, name="ones_col")
            ones8 = sb.tile([1, 8], f32, name="ones8")
            maskA = [sb.tile([P, 8], f32, name=f"maskA{t}", tag=f"maskA{t}") for t in range(4)]
            mkA = [sb.tile([P, 8], f32, name=f"mkA{t}", tag=f"mkA{t}") for t in range(4)]
            maskB = sb.tile([8, INNER], f32, name="maskB")
            colvT_sb = sb.tile([P, 4], f32, name="colvT_sb")
            ktvT_sb = sb.tile([P, 2 * P], f32, name="ktvT_sb")
            bd = [sb.tile([P, P], f32, name=f"bd{t}", tag=f"bd{t}") for t in range(4)]
            At_sb = [sb.tile([P, DIM], bf16, name=f"At{t}", tag=f"At{t}") for t in range(4)]
            B_sb = [sb.tile([P, P], bf16, name=f"Bt{t}", tag=f"Bt{t}") for t in range(4)]
            CCx = sb.tile([8, DIM], f32, name="CCx")
            CCq = sb.tile([8, INNER], f32, name="CCq")

            # ---------------- constants ----------------
            make_identity(nc, identf)
            make_identity(nc, identb)
            nc.gpsimd.memset(ones_col, 1.0)
            nc.gpsimd.memset(ones8, 1.0)
            for t in range(4):
                nc.gpsimd.memset(maskA[t], 0.0)
                nc.gpsimd.memset(maskA[t][0:64, 2 * t : 2 * t + 1], 1.0)
                nc.gpsimd.memset(maskA[t][64:P, 2 * t + 1 : 2 * t + 2], 1.0)
            nc.sync.dma_start(maskB, maskB_in[:, :])

            @contextmanager
            def backfill():
                save = tc.cur_priority
                tc.cur_priority = save + 1_000_000
                try:
                    yield
                finally:
                    tc.cur_priority = save

            # ---------------- helpers ----------------
            PRO_TAGS = ["dots", "dots", "pb", "u"]  # prologue rotates all psum slots
            pro_i = [0]

            def pro_tile(width, dtype, name):
                tag = PRO_TAGS[pro_i[0] % 4]
                pro_i[0] += 1
                return ps.tile(
                    [P, width], dtype, tag=tag,
                    bufs=(1 if tag in ("pb", "u") else None), name=name
                )

            def transpose_group(dst, src_tiles, d, g, name):
                # 4 [128,128] PE transposes packed into one [128,512] psum + 1 evac
                ptr = pro_tile(DIM, bf16, f"{name}{d}{g}")
                for k in range(4):
                    nc.tensor.transpose(
                        ptr[:, P * k : P * (k + 1)],
                        src_tiles[4 * g + k][:, P * d : P * (d + 1)],
                        identb,
                    )
                nc.vector.tensor_copy(dst[:, DIM * g : DIM * (g + 1)], ptr)

            def project_chunk(dst, wcol0, jc, name, psname=None):
                # dst[:, 512*jc:...] = w_qkv[:, wcol0:wcol0+128]^T @ x^T chunk
                kp = (
                    pro_tile(DIM, f32, f"{name}")
                    if psname is None
                    else ps.tile([P, DIM], f32, tag="u", bufs=1, name=f"{name}")
                )
                for d in range(4):
                    nc.tensor.matmul(
                        kp,
                        wqb[d][:, wcol0 : wcol0 + P],
                        xT[d][:, DIM * jc : DIM * (jc + 1)],
                        start=(d == 0),
                        stop=(d == 3),
                    )
                nc.vector.tensor_copy(dst[:, DIM * jc : DIM * (jc + 1)], kp)

            def dots_pair(it, s, h):
                col = 16 * it + 8 * s + h
                src = QT if s == 0 else q2T
                r0 = (h % 2) * DH
                lhsT = src[h // 2][r0 : r0 + DH, P * it : P * (it + 1)]
                dpa = ps.tile([P, NA], f32, tag="dots", name=f"dpa{col}")
                for jc in range(3):
                    nc.tensor.matmul(
                        dpa[:, DIM * jc : DIM * (jc + 1)],
                        lhsT,
                        KT[h // 2][r0 : r0 + DH, DIM * jc : DIM * (jc + 1)],
                        start=True,
                        stop=True,
                    )
                dpb = ps.tile([P, DIM], f32, tag="pb", bufs=1, name=f"dpb{col}")
                nc.tensor.matmul(
                    dpb,
                    lhsT,
                    KT[h // 2][r0 : r0 + DH, NA : N],
                    start=True,
                    stop=True,
                )
                escra = sb.tile([P, NA], bf16, tag="escra", bufs=2, name=f"ea{col}")
                nc.scalar.activation(
                    escra, dpa, AF.Exp, scale=SCALE,
                    accum_out=pa_all[:, col : col + 1],
                )
                escrb = sb.tile([P, DIM], bf16, tag="escrb", bufs=3, name=f"eb{col}")
                nc.scalar.activation(escrb, dpb, AF.Exp, scale=SCALE)
                nc.vector.reduce_sum(
                    pb_all[:, col : col + 1], escrb, axis=mybir.AxisListType.X
                )

            def kv_knp(j16):
                knp = ps.tile([P, DIM], f32, tag="u", bufs=1, name=f"knp{j16}")
                for d in range(4):
                    nc.tensor.matmul(
                        knp,
                        xT[d][:, P * j16 : P * (j16 + 1)],
                        wqb[d][:, INNER : 2 * INNER],
                        start=(d == 0), stop=(d == 3),
                    )
                kn_sb = sb.tile([P, DIM], bf16, tag="kn", bufs=2, name=f"kn{j16}")
                nc.vector.tensor_copy(kn_sb, knp)
                return kn_sb

            def kv_vnp(j16):
                vnp = ps.tile([P, DIM], f32, tag="u", bufs=1, name=f"vnp{j16}")
                for d in range(4):
                    nc.tensor.matmul(
                        vnp,
                        xT[d][:, P * j16 : P * (j16 + 1)],
                        wqb[d][:, 2 * INNER : 3 * INNER],
                        start=(d == 0), stop=(d == 3),
                    )
                vn_sb = sb.tile([P, DIM], bf16, tag="vn", bufs=2, name=f"vn{j16}")
                nc.vector.tensor_copy(vn_sb, vnp)
                return vn_sb

            def kv_ktv(j16, kn_sb, vn_sb):
                kvp = ps.tile([P, 2 * P], f32, tag="u", bufs=1, name=f"kvp{j16}")
                for h in range(HEADS):
                    nc.tensor.matmul(
                        kvp[(h % 2) * DH : (h % 2 + 1) * DH, DH * (h // 2) : DH * (h // 2 + 1)],
                        vn_sb[:, DH * h : DH * (h + 1)],
                        kn_sb[:, DH * h : DH * (h + 1)],
                        start=True, stop=True,
                    )
                cvp = ps.tile([1, INNER], f32, tag="u", bufs=1, name=f"cvp{j16}")
                nc.tensor.matmul(cvp, ones_col, vn_sb, start=True, stop=True)
                if j16 == 0:
                    nc.vector.tensor_copy(ktvT_acc, kvp[:, 0 : 2 * P])
                    nc.vector.tensor_copy(colv_acc, cvp)
                else:
                    nc.vector.tensor_add(ktvT_acc, ktvT_acc, kvp[:, 0 : 2 * P])
                    nc.vector.tensor_add(colv_acc, colv_acc, cvp)

            def finalize_ktv():
                nc.vector.tensor_scalar_mul(ktvT_sb, ktvT_acc, SCALE)
                for t in range(4):
                    nc.gpsimd.memset(bd[t], 0.0)
                    nc.vector.tensor_copy(bd[t][0:DH, 0:DH], ktvT_sb[0:DH, DH * t : DH * (t + 1)])
                    nc.vector.tensor_copy(bd[t][DH:P, DH:P], ktvT_sb[DH:P, DH * t : DH * (t + 1)])
                for t in range(4):
                    ap_ = ps.tile([P, DIM], f32, tag="u", bufs=1, name=f"ap{t}")
                    nc.tensor.matmul(ap_, bd[t], wo[t], start=True, stop=True)
                    nc.vector.tensor_copy(At_sb[t], ap_)
                for t in range(4):
                    bp = ps.tile([P, P], f32, tag="u", bufs=1, name=f"bp{t}")
                    nc.tensor.transpose(bp, bd[t], identf)
                    nc.vector.tensor_copy(B_sb[t], bp)
                cvt = ps.tile([P, 4], f32, tag="u", bufs=1, name="cvt")
                for t in range(4):
                    nc.tensor.matmul(
                        cvt[:, t : t + 1],
                        colv_acc[:, P * t : P * (t + 1)],
                        identf[0:1, 0:1],
                        start=True, stop=True,
                    )
                nc.vector.tensor_copy(colvT_sb, cvt)
                for t in range(4):
                    nc.vector.tensor_scalar_mul(mkA[t], maskA[t], colvT_sb[:, t : t + 1])
                cp = ps.tile([8, DIM], f32, tag="u", bufs=1, name="cp")
                for t in range(4):
                    nc.tensor.matmul(cp, mkA[t], wo[t], start=(t == 0), stop=(t == 3))
                nc.vector.tensor_scalar_mul(CCx, cp, -1.0)
                bc = ps.tile([8, INNER], f32, tag="u", bufs=1, name="bc")
                nc.tensor.matmul(bc, ones8, colv_acc, start=True, stop=True)
                nc.vector.tensor_mul(CCq, bc, maskB)

            def ln_batch(b):
                # fold partials + Ln for i-tiles [4b, 4b+4) -> cols 64b..64b+64
                c0, c1 = 64 * b, 64 * (b + 1)
                nc.vector.tensor_add(
                    se_all[:, c0:c1], pa_all[:, c0:c1], pb_all[:, c0:c1]
                )
                nc.scalar.activation(lse_all[:, c0:c1], se_all[:, c0:c1], AF.Ln)

            def assemble(it):
                ltx = ps.tile([8, P], f32, tag="u", bufs=1, name=f"ltx{it}")
                nc.tensor.transpose(ltx, lse_all[:, 16 * it : 16 * it + 8], identf)
                ltq = ps.tile([8, P], f32, tag="u", bufs=1, name=f"ltq{it}")
                nc.tensor.transpose(ltq, lse_all[:, 16 * it + 8 : 16 * it + 16], identf)
                lx_sb = sb.tile([8, P], f32, tag="lx", bufs=2, name=f"lx{it}")
                lq_sb = sb.tile([8, P], f32, tag="lq", bufs=2, name=f"lq{it}")
                nc.vector.tensor_copy(lx_sb, ltx)
                nc.vector.tensor_copy(lq_sb, ltq)

                xp = ps.tile([P, DIM], f32, tag="u", bufs=1, name=f"xp{it}")
                for t in range(4):
                    nc.tensor.matmul(
                        xp, QT[t][:, P * it : P * (it + 1)], At_sb[t],
                        start=(t == 0), stop=False,
                    )
                nc.tensor.matmul(xp, lx_sb, CCx, start=False, stop=True)
                xst = sb.tile([P, DIM], f32, tag="xst", bufs=2, name=f"xst{it}")
                nc.vector.tensor_add(xst, xp, xn[it])
                nc.sync.dma_start(xnew[P * it : P * (it + 1), :], xst)

                qp = ps.tile([P, INNER], f32, tag="u", bufs=1, name=f"qpo{it}")
                for t in range(4):
                    reg = qp[:, P * t : P * (t + 1)]
                    nc.tensor.matmul(
                        reg, lq_sb, CCq[:, P * t : P * (t + 1)], start=True, stop=False
                    )
                    nc.tensor.matmul(
                        reg, q2T[t][:, P * it : P * (it + 1)], B_sb[t],
                        start=False, stop=True,
                    )
                qst = sb.tile([P, INNER], f32, tag="qst", bufs=2, name=f"qst{it}")
                nc.vector.tensor_add(qst, qp, q2n[it])
                nc.sync.dma_start(qnew[P * it : P * (it + 1), :], qst)

            # ---------------- prologue: finely interleaved ----------------
            for d in range(4):
                nc.sync.dma_start(wq[d], w_qkv[P * d : P * (d + 1), :])
                nc.vector.tensor_copy(wqb[d], wq[d])
            for g in range(4):
                for k in range(4):
                    j = 4 * g + k
                    nc.sync.dma_start(xn[j], x_b[P * j : P * (j + 1), :])
                    nc.vector.tensor_copy(xnb[j], xn[j])
                for d in range(4):
                    transpose_group(xT[d], xnb, d, g, "tx")
                # KT0 chunk g ready as soon as its xT columns exist
                project_chunk(KT[0], INNER, g, f"kp0{g}")
                if g < 2:
                    project_chunk(QT[0], 0, g, f"qp0{g}")
            for d in range(4):
                nc.sync.dma_start(wo[d], w_out[P * d : P * (d + 1), :])
            for j in range(NIT):
                nc.sync.dma_start(q2n[j], qoir_r[P * j : P * (j + 1), :])
                nc.vector.tensor_copy(q2nb[j], q2n[j])
            for g in range(2):
                for d in range(4):
                    transpose_group(q2T[d], q2nb, d, g, "tq")

            # late projections: backfill
            with backfill():
                for t in range(1, 4):
                    for jc in range(4):
                        project_chunk(KT[t], INNER + P * t, jc, f"kp{t}{jc}", psname="u")
                    for ic in range(2):
                        project_chunk(QT[t], P * t, ic, f"qq{t}{ic}", psname="u")

            # ---------------- dots stream with utility backfill ----------------
            kn_live = {}
            vn_live = {}
            for t in range(4):
                for it in range(NIT):
                    for s in range(2):
                        for h in (2 * t, 2 * t + 1):
                            dots_pair(it, s, h)
                    slot = 8 * t + it
                    if slot < 16:
                        with backfill():
                            kn = kv_knp(slot)
                            vn = kv_vnp(slot)
                            kv_ktv(slot, kn, vn)
                    if slot == 17:
                        with backfill():
                            finalize_ktv()
                    if t == 3 and it == 4:
                        with backfill():
                            ln_batch(0)
                            for a_it in range(4):
                                assemble(a_it)

            # ---------------- tail ----------------
            with backfill():
                ln_batch(1)
                for a_it in range(4, NIT):
                    assemble(a_it)

    nc.compile()
    return nc


_CACHE = {}


def _get_nc():
    if "nc" not in _CACHE:
        _CACHE["nc"] = build_bass()
    return _CACHE["nc"]


def _shard_inputs(x, qoir):
    """Per-core input maps. Core c: batch c//2, row-half c%2, own rows first."""
    in_maps = []
    for c in range(NCORES):
        b, half = c // 2, c % 2
        mine = x[b, half * ROWS : (half + 1) * ROWS]
        other = x[b, (1 - half) * ROWS : (2 - half) * ROWS]
        in_maps.append(
            {
                "x_b": np.ascontiguousarray(np.concatenate([mine, other], axis=0)),
                "qoir_r": np.ascontiguousarray(qoir[b, half * ROWS : (half + 1) * ROWS]),
            }
        )
    return in_maps


def _maskB():
    mb = np.zeros((8, INNER), dtype=np.float32)
    for h in range(8):
        mb[h, DH * h : DH * (h + 1)] = -1.0
    return mb


def kernel(x, qoir, w_qkv, w_out):
    from concourse.bass_utils import run_bass_kernel_spmd

    x = np.asarray(x, dtype=np.float32)
    qoir = np.asarray(qoir, dtype=np.float32)
    w_qkv = np.ascontiguousarray(np.asarray(w_qkv, dtype=np.float32))
    w_out = np.ascontiguousarray(np.asarray(w_out, dtype=np.float32))

    nc = _get_nc()
    in_maps = _shard_inputs(x, qoir)
    for m in in_maps:
        m["w_qkv"] = w_qkv
        m["w_out"] = w_out
        m["maskB_in"] = _maskB()

    res = run_bass_kernel_spmd(nc, in_maps, core_ids=list(range(NCORES)))
    x_new = np.empty((B, N, DIM), dtype=np.float32)
    q_new = np.empty((B, N, INNER), dtype=np.float32)
    for c in range(NCORES):
        b, half = c // 2, c % 2
        rows = slice(half * ROWS, (half + 1) * ROWS)
        x_new[b, rows] = res.results[c]["xnew_p"]
        q_new[b, rows] = res.results[c]["qnew_p"]
    return (x_new, q_new)


# revision 25
# speedup vs baseline: 1.3986x; 1.0704x over previous
"""Trainium2 Bass kernel for the LogSoftmax dual-stream attention module.

Math (per batch b, head h):
    qkv = x @ w_qkv ; q,k,v = split(qkv); q2 = qoir
    dots  = scale * q  @ k^T ; dots2 = scale * q2 @ k^T
    attn  = log_softmax(dots) = scale*dots_raw - lse       (log-probs!)
    out   = attn @ v  = scale * q @ (k^T v) - lse  (x) colsum(v)
    qout  = attn2 @ v = scale * q2 @ (k^T v) - lse2 (x) colsum(v)
    x_new = merge(out) @ w_out + x ; q_new = merge(qout) + qoir

The factorization removes the O(N^2) attn@V matmul entirely; the only O(N^2)
work is lse = ln(rowsum(exp(dots))), computed on ScalarE with the fused
activation accum_out (exp + row-sum in one instruction), dots on TensorE.

Sharding: 8 cores = (batch 0..3) x (row-half 0..1). Each core gets the full
2048 keys of its batch (rows permuted so its own 1024 query rows come first —
all key-side reductions are permutation invariant), computes its 1024 rows of
both outputs. No collectives.
"""

import numpy as np

B, N, DIM = 4, 2048, 512
HEADS, DH = 8, 64
INNER = HEADS * DH          # 512
ROWS = N // 2               # 1024 query rows per core
SCALE = DH ** -0.5          # 0.125
NCORES = 8

P = 128                     # partitions
NJT = N // P                # 16 key j-tiles
NIT = ROWS // P             # 8 query i-tiles


def build_bass():
    import concourse.bass as bass
    import concourse.mybir as mybir
    import concourse.tile as tile
    from concourse import bacc
    from concourse.masks import make_identity
    from contextlib import contextmanager

    f32 = mybir.dt.float32
    bf16 = mybir.dt.bfloat16
    AF = mybir.ActivationFunctionType

    nc = bacc.Bacc()

    x_b = nc.declare_dram_parameter("x_b", [N, DIM], f32, isOutput=False)
    qoir_r = nc.declare_dram_parameter("qoir_r", [ROWS, INNER], f32, isOutput=False)
    w_qkv = nc.declare_dram_parameter("w_qkv", [DIM, 3 * INNER], f32, isOutput=False)
    w_out = nc.declare_dram_parameter("w_out", [INNER, DIM], f32, isOutput=False)
    maskB_in = nc.declare_dram_parameter("maskB_in", [8, INNER], f32, isOutput=False)
    xnew = nc.declare_dram_parameter("xnew_p", [ROWS, DIM], f32, isOutput=True)
    qnew = nc.declare_dram_parameter("qnew_p", [ROWS, INNER], f32, isOutput=True)

    NA = 3 * DIM  # 1536: EXP part A width

    with tile.TileContext(nc) as tc:
        with (
            tc.tile_pool(name="sb", bufs=1) as sb,
            tc.tile_pool(name="ps", bufs=2, space="PSUM") as ps,
        ):
            # ---------------- persistent SBUF ----------------
            wq = [sb.tile([P, 3 * INNER], f32, name=f"wq{d}", tag=f"wq{d}") for d in range(4)]
            wqb = [sb.tile([P, 3 * INNER], bf16, name=f"wqb{d}", tag=f"wqb{d}") for d in range(4)]
            wo = [sb.tile([P, DIM], f32, name=f"wo{d}", tag=f"wo{d}") for d in range(4)]
            xn = [
                sb.tile([P, DIM], f32, name=f"xn{j}", tag=f"xn{j}")
                if j < NIT
                else sb.tile([P, DIM], f32, name=f"xn{j}", tag="xnrot", bufs=3)
                for j in range(NJT)
            ]
            xnb = [sb.tile([P, DIM], bf16, name=f"xnb{j}", tag=f"xnb{j}") for j in range(NJT)]
            q2n = [sb.tile([P, INNER], f32, name=f"q2n{j}", tag=f"q2n{j}") for j in range(NIT)]
            q2nb = [sb.tile([P, INNER], bf16, name=f"q2nb{j}", tag=f"q2nb{j}") for j in range(NIT)]
            xT = [sb.tile([P, N], bf16, name=f"xT{d}", tag=f"xT{d}") for d in range(4)]
            q2T = [sb.tile([P, ROWS], bf16, name=f"q2T{d}", tag=f"q2T{d}") for d in range(4)]
            QT = [sb.tile([P, ROWS], bf16, name=f"QT{t}", tag=f"QT{t}") for t in range(4)]
            KT = [sb.tile([P, N], bf16, name=f"KT{t}", tag=f"KT{t}") for t in range(4)]
            ktvT_acc = sb.tile([P, 2 * P], f32, name="ktvT_acc")
            colv_acc = sb.tile([1, INNER], f32, name="colv_acc")
            pa_all = sb.tile([P, P], f32, name="pa_all")
            pb_all = sb.tile([P, P], f32, name="pb_all")
            se_all = sb.tile([P, P], f32, name="se_all")
            lse_all = sb.tile([P, P], f32, name="lse_all")
            identf = sb.tile([P, P], f32, name="identf")
            identb = sb.tile([P, P], bf16, name="identb")
            ones_col = sb.tile([P, 1], bf16, name="ones_col")
            ones8 = sb.tile([1, 8], f32, name="ones8")
            maskA = [sb.tile([P, 8], f32, name=f"maskA{t}", tag=f"maskA{t}") for t in range(4)]
            mkA = [sb.tile([P, 8], f32, name=f"mkA{t}", tag=f"mkA{t}") for t in range(4)]
            maskB = sb.tile([8, INNER], f32, name="maskB")
            colvT_sb = sb.tile([P, 4], f32, name="colvT_sb")
            ktvT_sb = sb.tile([P, 2 * P], f32, name="ktvT_sb")
            bd = [sb.tile([P, P], f32, name=f"bd{t}", tag=f"bd{t}") for t in range(4)]
            At_sb = [sb.tile([P, DIM], bf16, name=f"At{t}", tag=f"At{t}") for t in range(4)]
            B_sb = [sb.tile([P, P], bf16, name=f"Bt{t}", tag=f"Bt{t}") for t in range(4)]
            CCx = sb.tile([8, DIM], f32, name="CCx")
            CCq = sb.tile([8, INNER], f32, name="CCq")

            # ---------------- constants ----------------
            make_identity(nc, identf)
            make_identity(nc, identb)
            nc.gpsimd.memset(ones_col, 1.0)
            nc.gpsimd.memset(ones8, 1.0)
            for t in range(4):
                nc.gpsimd.memset(maskA[t], 0.0)
                nc.gpsimd.memset(maskA[t][0:64, 2 * t : 2 * t + 1], 1.0)
                nc.gpsimd.memset(maskA[t][64:P, 2 * t + 1 : 2 * t + 2], 1.0)
            nc.sync.dma_start(maskB, maskB_in[:, :])

            @contextmanager
            def backfill():
                save = tc.cur_priority
                tc.cur_priority = save + 1_000_000
                try:
                    yield
                finally:
                    tc.cur_priority = save

            # ---------------- helpers ----------------
            PRO_TAGS = ["dots", "dots", "pb", "u"]  # prologue rotates all psum slots
            pro_i = [0]

            def pro_tile(width, dtype, name):
                tag = PRO_TAGS[pro_i[0] % 4]
                pro_i[0] += 1
                return ps.tile(
                    [P, width], dtype, tag=tag,
                    bufs=(1 if tag in ("pb", "u") else None), name=name
                )

            def transpose_group(dst, src_tiles, d, g, name):
                # 4 [128,128] PE transposes packed into one [128,512] psum + 1 evac
                ptr = pro_tile(DIM, bf16, f"{name}{d}{g}")
                for k in range(4):
                    nc.tensor.transpose(
                        ptr[:, P * k : P * (k + 1)],
                        src_tiles[4 * g + k][:, P * d : P * (d + 1)],
                        identb,
                    )
                nc.vector.tensor_copy(dst[:, DIM * g : DIM * (g + 1)], ptr)

            def project_chunk(dst, wcol0, jc, name, psname=None):
                # dst[:, 512*jc:...] = w_qkv[:, wcol0:wcol0+128]^T @ x^T chunk
                kp = (
                    pro_tile(DIM, f32, f"{name}")
                    if psname is None
                    else ps.tile([P, DIM], f32, tag="u", bufs=1, name=f"{name}")
                )
                for d in range(4):
                    nc.tensor.matmul(
                        kp,
                        wqb[d][:, wcol0 : wcol0 + P],
                        xT[d][:, DIM * jc : DIM * (jc + 1)],
                        start=(d == 0),
                        stop=(d == 3),
                    )
                nc.vector.tensor_copy(dst[:, DIM * jc : DIM * (jc + 1)], kp)

            def dots_pair(it, s, h):
                col = 16 * it + 8 * s + h
                src = QT if s == 0 else q2T
                r0 = (h % 2) * DH
                lhsT = src[h // 2][r0 : r0 + DH, P * it : P * (it + 1)]
                dpa = ps.tile([P, NA], f32, tag="dots", name=f"dpa{col}")
                for jc in range(3):
                    nc.tensor.matmul(
                        dpa[:, DIM * jc : DIM * (jc + 1)],
                        lhsT,
                        KT[h // 2][r0 : r0 + DH, DIM * jc : DIM * (jc + 1)],
                        start=True,
                        stop=True,
                    )
                dpb = ps.tile([P, DIM], f32, tag="pb", bufs=1, name=f"dpb{col}")
                nc.tensor.matmul(
                    dpb,
                    lhsT,
                    KT[h // 2][r0 : r0 + DH, NA : N],
                    start=True,
                    stop=True,
                )
                escra = sb.tile([P, NA], bf16, tag="escra", bufs=2, name=f"ea{col}")
                nc.scalar.activation(
                    escra, dpa, AF.Exp, scale=SCALE,
                    accum_out=pa_all[:, col : col + 1],
                )
                escrb = sb.tile([P, DIM], bf16, tag="escrb", bufs=3, name=f"eb{col}")
                nc.scalar.activation(escrb, dpb, AF.Exp, scale=SCALE)
                nc.vector.reduce_sum(
                    pb_all[:, col : col + 1], escrb, axis=mybir.AxisListType.X
                )

            def kv_knp(j16):
                knp = ps.tile([P, DIM], f32, tag="u", bufs=1, name=f"knp{j16}")
                for d in range(4):
                    nc.tensor.matmul(
                        knp,
                        xT[d][:, P * j16 : P * (j16 + 1)],
                        wqb[d][:, INNER : 2 * INNER],
                        start=(d == 0), stop=(d == 3),
                    )
                kn_sb = sb.tile([P, DIM], bf16, tag="kn", bufs=2, name=f"kn{j16}")
                nc.vector.tensor_copy(kn_sb, knp)
                return kn_sb

            def kv_vnp(j16):
                vnp = ps.tile([P, DIM], f32, tag="u", bufs=1, name=f"vnp{j16}")
                for d in range(4):
                    nc.tensor.matmul(
                        vnp,
                        xT[d][:, P * j16 : P * (j16 + 1)],
                        wqb[d][:, 2 * INNER : 3 * INNER],
                        start=(d == 0), stop=(d == 3),
                    )
                vn_sb = sb.tile([P, DIM], bf16, tag="vn", bufs=2, name=f"vn{j16}")
                nc.vector.tensor_copy(vn_sb, vnp)
                return vn_sb

            def kv_ktv(j16, kn_sb, vn_sb):
                kvp = ps.tile([P, 2 * P], f32, tag="u", bufs=1, name=f"kvp{j16}")
                for h in range(HEADS):
                    nc.tensor.matmul(
                        kvp[(h % 2) * DH : (h % 2 + 1) * DH, DH * (h // 2) : DH * (h // 2 + 1)],
                        vn_sb[:, DH * h : DH * (h + 1)],
                        kn_sb[:, DH * h : DH * (h + 1)],
                        start=True, stop=True,
                    )
                cvp = ps.tile([1, INNER], f32, tag="u", bufs=1, name=f"cvp{j16}")
                nc.tensor.matmul(cvp, ones_col, vn_sb, start=True, stop=True)
                if j16 == 0:
                    nc.vector.tensor_copy(ktvT_acc, kvp[:, 0 : 2 * P])
                    nc.vector.tensor_copy(colv_acc, cvp)
                else:
                    nc.vector.tensor_add(ktvT_acc, ktvT_acc, kvp[:, 0 : 2 * P])
                    nc.vector.tensor_add(colv_acc, colv_acc, cvp)

            def finalize_ktv():
                nc.vector.tensor_scalar_mul(ktvT_sb, ktvT_acc, SCALE)
                for t in range(4):
                    nc.gpsimd.memset(bd[t], 0.0)
                    nc.vector.tensor_copy(bd[t][0:DH, 0:DH], ktvT_sb[0:DH, DH * t : DH * (t + 1)])
                    nc.vector.tensor_copy(bd[t][DH:P, DH:P], ktvT_sb[DH:P, DH * t : DH * (t + 1)])
                for t in range(4):
                    ap_ = ps.tile([P, DIM], f32, tag="u", bufs=1, name=f"ap{t}")
                    nc.tensor.matmul(ap_, bd[t], wo[t], start=True, stop=True)
                    nc.vector.tensor_copy(At_sb[t], ap_)
                for t in range(4):
                    bp = ps.tile([P, P], f32, tag="u", bufs=1, name=f"bp{t}")
                    nc.tensor.transpose(bp, bd[t], identf)
                    nc.vector.tensor_copy(B_sb[t], bp)
                cvt = ps.tile([P, 4], f32, tag="u", bufs=1, name="cvt")
                for t in range(4):
                    nc.tensor.matmul(
                        cvt[:, t : t + 1],
                        colv_acc[:, P * t : P * (t + 1)],
                        identf[0:1, 0:1],
                        start=True, stop=True,
                    )
                nc.vector.tensor_copy(colvT_sb, cvt)
                for t in range(4):
                    nc.vector.tensor_scalar_mul(mkA[t], maskA[t], colvT_sb[:, t : t + 1])
                cp = ps.tile([8, DIM], f32, tag="u", bufs=1, name="cp")
                for t in range(4):
                    nc.tensor.matmul(cp, mkA[t], wo[t], start=(t == 0), stop=(t == 3))
                nc.vector.tensor_scalar_mul(CCx, cp, -1.0)
                bc = ps.tile([8, INNER], f32, tag="u", bufs=1, name="bc")
                nc.tensor.matmul(bc, ones8, colv_acc, start=True, stop=True)
                nc.vector.tensor_mul(CCq, bc, maskB)

            def ln_batch(b):
                # fold partials + Ln for i-tiles [4b, 4b+4) -> cols 64b..64b+64
                c0, c1 = 64 * b, 64 * (b + 1)
                nc.vector.tensor_add(
                    se_all[:, c0:c1], pa_all[:, c0:c1], pb_all[:, c0:c1]
                )
                nc.scalar.activation(lse_all[:, c0:c1], se_all[:, c0:c1], AF.Ln)

            def assemble(it, tags=("u",)):
                def atile(width, name):
                    tag = tags[atile.i % len(tags)]
                    atile.i += 1
                    return ps.tile(
                        [P, width], f32, tag=tag,
                        bufs=(1 if tag in ("pb", "u") else None), name=name
                    )
                atile.i = 0
                ltx = atile(P, f"ltx{it}")[0:8, :]
                nc.tensor.transpose(ltx, lse_all[:, 16 * it : 16 * it + 8], identf)
                ltq = atile(P, f"ltq{it}")[0:8, :]
                nc.tensor.transpose(ltq, lse_all[:, 16 * it + 8 : 16 * it + 16], identf)
                lx_sb = sb.tile([8, P], f32, tag="lx", bufs=2, name=f"lx{it}")
                lq_sb = sb.tile([8, P], f32, tag="lq", bufs=2, name=f"lq{it}")
                nc.vector.tensor_copy(lx_sb, ltx)
                nc.vector.tensor_copy(lq_sb, ltq)

                xp = atile(DIM, f"xp{it}")
                for t in range(4):
                    nc.tensor.matmul(
                        xp, QT[t][:, P * it : P * (it + 1)], At_sb[t],
                        start=(t == 0), stop=False,
                    )
                nc.tensor.matmul(xp, lx_sb, CCx, start=False, stop=True)
                xst = sb.tile([P, DIM], f32, tag="xst", bufs=2, name=f"xst{it}")
                nc.vector.tensor_add(xst, xp, xn[it])
                nc.sync.dma_start(xnew[P * it : P * (it + 1), :], xst)

                qp = atile(INNER, f"qpo{it}")
                for t in range(4):
                    reg = qp[:, P * t : P * (t + 1)]
                    nc.tensor.matmul(
                        reg, lq_sb, CCq[:, P * t : P * (t + 1)], start=True, stop=False
                    )
                    nc.tensor.matmul(
                        reg, q2T[t][:, P * it : P * (it + 1)], B_sb[t],
                        start=False, stop=True,
                    )
                qst = sb.tile([P, INNER], f32, tag="qst", bufs=2, name=f"qst{it}")
                nc.vector.tensor_add(qst, qp, q2n[it])
                nc.sync.dma_start(qnew[P * it : P * (it + 1), :], qst)

            # ---------------- prologue: finely interleaved ----------------
            for d in range(4):
                for c0 in (INNER, 0, 2 * INNER):  # K cols first, then Q, then V
                    nc.sync.dma_start(
                        wq[d][:, c0 : c0 + INNER],
                        w_qkv[P * d : P * (d + 1), c0 : c0 + INNER],
                    )
                    nc.vector.tensor_copy(
                        wqb[d][:, c0 : c0 + INNER], wq[d][:, c0 : c0 + INNER]
                    )
            for g in range(4):
                for k in range(4):
                    j = 4 * g + k
                    nc.sync.dma_start(xn[j], x_b[P * j : P * (j + 1), :])
                    nc.vector.tensor_copy(xnb[j], xn[j])
                for d in range(4):
                    transpose_group(xT[d], xnb, d, g, "tx")
                # KT0 chunk g ready as soon as its xT columns exist
                project_chunk(KT[0], INNER, g, f"kp0{g}")
                if g < 2:
                    project_chunk(QT[0], 0, g, f"qp0{g}")
            for d in range(4):
                nc.sync.dma_start(wo[d], w_out[P * d : P * (d + 1), :])
            for j in range(NIT):
                nc.sync.dma_start(q2n[j], qoir_r[P * j : P * (j + 1), :])
                nc.vector.tensor_copy(q2nb[j], q2n[j])
            for g in range(2):
                for d in range(4):
                    transpose_group(q2T[d], q2nb, d, g, "tq")

            # late projections: backfill
            with backfill():
                for t in range(1, 4):
                    for jc in range(4):
                        project_chunk(KT[t], INNER + P * t, jc, f"kp{t}{jc}", psname="u")
                    for ic in range(2):
                        project_chunk(QT[t], P * t, ic, f"qq{t}{ic}", psname="u")

            # ---------------- dots stream with utility backfill ----------------
            kn_live = {}
            vn_live = {}
            for t in range(4):
                for it in range(NIT):
                    for s in range(2):
                        for h in (2 * t, 2 * t + 1):
                            dots_pair(it, s, h)
                    slot = 8 * t + it
                    if slot < 16:
                        with backfill():
                            kn = kv_knp(slot)
                            vn = kv_vnp(slot)
                            kv_ktv(slot, kn, vn)
                    if slot == 17:
                        with backfill():
                            finalize_ktv()
                    if t == 3 and it == 4:
                        ln_batch(0)
                        with backfill():
                            for a_it in range(4):
                                assemble(a_it)

            # ---------------- tail ----------------
            ln_batch(1)
            for a_it in range(4, NIT):
                assemble(a_it, tags=("u", "pb", "dots", "dots"))

    nc.compile()
    return nc


_CACHE = {}


def _get_nc():
    if "nc" not in _CACHE:
        _CACHE["nc"] = build_bass()
    return _CACHE["nc"]


def _shard_inputs(x, qoir):
    """Per-core input maps. Core c: batch c//2, row-half c%2, own rows first."""
    in_maps = []
    for c in range(NCORES):
        b, half = c // 2, c % 2
        mine = x[b, half * ROWS : (half + 1) * ROWS]
        other = x[b, (1 - half) * ROWS : (2 - half) * ROWS]
        in_maps.append(
            {
                "x_b": np.ascontiguousarray(np.concatenate([mine, other], axis=0)),
                "qoir_r": np.ascontiguousarray(qoir[b, half * ROWS : (half + 1) * ROWS]),
            }
        )
    return in_maps


def _maskB():
    mb = np.zeros((8, INNER), dtype=np.float32)
    for h in range(8):
        mb[h, DH * h : DH * (h + 1)] = -1.0
    return mb


def kernel(x, qoir, w_qkv, w_out):
    from concourse.bass_utils import run_bass_kernel_spmd

    x = np.asarray(x, dtype=np.float32)
    qoir = np.asarray(qoir, dtype=np.float32)
    w_qkv = np.ascontiguousarray(np.asarray(w_qkv, dtype=np.float32))
    w_out = np.ascontiguousarray(np.asarray(w_out, dtype=np.float32))

    nc = _get_nc()
    in_maps = _shard_inputs(x, qoir)
    for m in in_maps:
        m["w_qkv"] = w_qkv
        m["w_out"] = w_out
        m["maskB_in"] = _maskB()

    res = run_bass_kernel_spmd(nc, in_maps, core_ids=list(range(NCORES)))
    x_new = np.empty((B, N, DIM), dtype=np.float32)
    q_new = np.empty((B, N, INNER), dtype=np.float32)
    for c in range(NCORES):
        b, half = c // 2, c % 2
        rows = slice(half * ROWS, (half + 1) * ROWS)
        x_new[b, rows] = res.results[c]["xnew_p"]
        q_new[b, rows] = res.results[c]["qnew_p"]
    return (x_new, q_new)


# revision 26
# speedup vs baseline: 1.4039x; 1.0038x over previous
"""Trainium2 Bass kernel for the LogSoftmax dual-stream attention module.

Math (per batch b, head h):
    qkv = x @ w_qkv ; q,k,v = split(qkv); q2 = qoir
    dots  = scale * q  @ k^T ; dots2 = scale * q2 @ k^T
    attn  = log_softmax(dots) = scale*dots_raw - lse       (log-probs!)
    out   = attn @ v  = scale * q @ (k^T v) - lse  (x) colsum(v)
    qout  = attn2 @ v = scale * q2 @ (k^T v) - lse2 (x) colsum(v)
    x_new = merge(out) @ w_out + x ; q_new = merge(qout) + qoir

The factorization removes the O(N^2) attn@V matmul entirely; the only O(N^2)
work is lse = ln(rowsum(exp(dots))), computed on ScalarE with the fused
activation accum_out (exp + row-sum in one instruction), dots on TensorE.

Sharding: 8 cores = (batch 0..3) x (row-half 0..1). Each core gets the full
2048 keys of its batch (rows permuted so its own 1024 query rows come first —
all key-side reductions are permutation invariant), computes its 1024 rows of
both outputs. No collectives.
"""

import numpy as np

B, N, DIM = 4, 2048, 512
HEADS, DH = 8, 64
INNER = HEADS * DH          # 512
ROWS = N // 2               # 1024 query rows per core
SCALE = DH ** -0.5          # 0.125
NCORES = 8

P = 128                     # partitions
NJT = N // P                # 16 key j-tiles
NIT = ROWS // P             # 8 query i-tiles


def build_bass():
    import concourse.bass as bass
    import concourse.mybir as mybir
    import concourse.tile as tile
    from concourse import bacc
    from concourse.masks import make_identity
    from contextlib import contextmanager

    f32 = mybir.dt.float32
    bf16 = mybir.dt.bfloat16
    AF = mybir.ActivationFunctionType

    nc = bacc.Bacc()

    x_b = nc.declare_dram_parameter("x_b", [N, DIM], f32, isOutput=False)
    qoir_r = nc.declare_dram_parameter("qoir_r", [ROWS, INNER], f32, isOutput=False)
    w_qkv = nc.declare_dram_parameter("w_qkv", [DIM, 3 * INNER], f32, isOutput=False)
    w_out = nc.declare_dram_parameter("w_out", [INNER, DIM], f32, isOutput=False)
    maskB_in = nc.declare_dram_parameter("maskB_in", [8, INNER], f32, isOutput=False)
    ident_in = nc.declare_dram_parameter("ident_in", [P, P], f32, isOutput=False)
    xnew = nc.declare_dram_parameter("xnew_p", [ROWS, DIM], f32, isOutput=True)
    qnew = nc.declare_dram_parameter("qnew_p", [ROWS, INNER], f32, isOutput=True)

    NA = 3 * DIM  # 1536: EXP part A width

    with tile.TileContext(nc) as tc:
        with (
            tc.tile_pool(name="sb", bufs=1) as sb,
            tc.tile_pool(name="ps", bufs=2, space="PSUM") as ps,
        ):
            # ---------------- persistent SBUF ----------------
            wq = [sb.tile([P, 3 * INNER], f32, name=f"wq{d}", tag=f"wq{d}") for d in range(4)]
            wqb = [sb.tile([P, 3 * INNER], bf16, name=f"wqb{d}", tag=f"wqb{d}") for d in range(4)]
            wo = [sb.tile([P, DIM], f32, name=f"wo{d}", tag=f"wo{d}") for d in range(4)]
            xn = [
                sb.tile([P, DIM], f32, name=f"xn{j}", tag=f"xn{j}")
                if j < NIT
                else sb.tile([P, DIM], f32, name=f"xn{j}", tag="xnrot", bufs=3)
                for j in range(NJT)
            ]
            xnb = [sb.tile([P, DIM], bf16, name=f"xnb{j}", tag=f"xnb{j}") for j in range(NJT)]
            q2n = [sb.tile([P, INNER], f32, name=f"q2n{j}", tag=f"q2n{j}") for j in range(NIT)]
            q2nb = [sb.tile([P, INNER], bf16, name=f"q2nb{j}", tag=f"q2nb{j}") for j in range(NIT)]
            xT = [sb.tile([P, N], bf16, name=f"xT{d}", tag=f"xT{d}") for d in range(4)]
            q2T = [sb.tile([P, ROWS], bf16, name=f"q2T{d}", tag=f"q2T{d}") for d in range(4)]
            QT = [sb.tile([P, ROWS], bf16, name=f"QT{t}", tag=f"QT{t}") for t in range(4)]
            KT = [sb.tile([P, N], bf16, name=f"KT{t}", tag=f"KT{t}") for t in range(4)]
            ktvT_acc = sb.tile([P, 2 * P], f32, name="ktvT_acc")
            colv_acc = sb.tile([1, INNER], f32, name="colv_acc")
            pa_all = sb.tile([P, P], f32, name="pa_all")
            pb_all = sb.tile([P, P], f32, name="pb_all")
            se_all = sb.tile([P, P], f32, name="se_all")
            lse_all = sb.tile([P, P], f32, name="lse_all")
            identf = sb.tile([P, P], f32, name="identf")
            identb = sb.tile([P, P], bf16, name="identb")
            ones_col = sb.tile([P, 1], bf16, name="ones_col")
            ones8 = sb.tile([1, 8], f32, name="ones8")
            maskA = [sb.tile([P, 8], f32, name=f"maskA{t}", tag=f"maskA{t}") for t in range(4)]
            mkA = [sb.tile([P, 8], f32, name=f"mkA{t}", tag=f"mkA{t}") for t in range(4)]
            maskB = sb.tile([8, INNER], f32, name="maskB")
            colvT_sb = sb.tile([P, 4], f32, name="colvT_sb")
            ktvT_sb = sb.tile([P, 2 * P], f32, name="ktvT_sb")
            bd = [sb.tile([P, P], f32, name=f"bd{t}", tag=f"bd{t}") for t in range(4)]
            At_sb = [sb.tile([P, DIM], bf16, name=f"At{t}", tag=f"At{t}") for t in range(4)]
            B_sb = [sb.tile([P, P], bf16, name=f"Bt{t}", tag=f"Bt{t}") for t in range(4)]
            CCx = sb.tile([8, DIM], f32, name="CCx")
            CCq = sb.tile([8, INNER], f32, name="CCq")

            # ---------------- constants ----------------
            nc.sync.dma_start(identf, ident_in[:, :])
            nc.vector.tensor_copy(identb, identf)
            nc.gpsimd.memset(ones_col, 1.0)
            nc.gpsimd.memset(ones8, 1.0)
            for t in range(4):
                nc.gpsimd.memset(maskA[t], 0.0)
                nc.gpsimd.memset(maskA[t][0:64, 2 * t : 2 * t + 1], 1.0)
                nc.gpsimd.memset(maskA[t][64:P, 2 * t + 1 : 2 * t + 2], 1.0)
            nc.sync.dma_start(maskB, maskB_in[:, :])

            @contextmanager
            def backfill():
                save = tc.cur_priority
                tc.cur_priority = save + 1_000_000
                try:
                    yield
                finally:
                    tc.cur_priority = save

            # ---------------- helpers ----------------
            PRO_TAGS = ["dots", "dots", "pb", "u"]  # prologue rotates all psum slots
            pro_i = [0]

            def pro_tile(width, dtype, name):
                tag = PRO_TAGS[pro_i[0] % 4]
                pro_i[0] += 1
                return ps.tile(
                    [P, width], dtype, tag=tag,
                    bufs=(1 if tag in ("pb", "u") else None), name=name
                )

            def transpose_group(dst, src_tiles, d, g, name):
                # 4 [128,128] PE transposes packed into one [128,512] psum + 1 evac
                ptr = pro_tile(DIM, bf16, f"{name}{d}{g}")
                for k in range(4):
                    nc.tensor.transpose(
                        ptr[:, P * k : P * (k + 1)],
                        src_tiles[4 * g + k][:, P * d : P * (d + 1)],
                        identb,
                    )
                nc.vector.tensor_copy(dst[:, DIM * g : DIM * (g + 1)], ptr)

            def project_chunk(dst, wcol0, jc, name, psname=None):
                # dst[:, 512*jc:...] = w_qkv[:, wcol0:wcol0+128]^T @ x^T chunk
                kp = (
                    pro_tile(DIM, f32, f"{name}")
                    if psname is None
                    else ps.tile([P, DIM], f32, tag="u", bufs=1, name=f"{name}")
                )
                for d in range(4):
                    nc.tensor.matmul(
                        kp,
                        wqb[d][:, wcol0 : wcol0 + P],
                        xT[d][:, DIM * jc : DIM * (jc + 1)],
                        start=(d == 0),
                        stop=(d == 3),
                    )
                nc.vector.tensor_copy(dst[:, DIM * jc : DIM * (jc + 1)], kp)

            def dots_pair(it, s, h):
                col = 16 * it + 8 * s + h
                src = QT if s == 0 else q2T
                r0 = (h % 2) * DH
                lhsT = src[h // 2][r0 : r0 + DH, P * it : P * (it + 1)]
                dpa = ps.tile([P, NA], f32, tag="dots", name=f"dpa{col}")
                for jc in range(3):
                    nc.tensor.matmul(
                        dpa[:, DIM * jc : DIM * (jc + 1)],
                        lhsT,
                        KT[h // 2][r0 : r0 + DH, DIM * jc : DIM * (jc + 1)],
                        start=True,
                        stop=True,
                    )
                dpb = ps.tile([P, DIM], f32, tag="pb", bufs=1, name=f"dpb{col}")
                nc.tensor.matmul(
                    dpb,
                    lhsT,
                    KT[h // 2][r0 : r0 + DH, NA : N],
                    start=True,
                    stop=True,
                )
                escra = sb.tile([P, NA], bf16, tag="escra", bufs=2, name=f"ea{col}")
                nc.scalar.activation(
                    escra, dpa, AF.Exp, scale=SCALE,
                    accum_out=pa_all[:, col : col + 1],
                )
                escrb = sb.tile([P, DIM], bf16, tag="escrb", bufs=6, name=f"eb{col}")
                nc.scalar.activation(escrb, dpb, AF.Exp, scale=SCALE)
                nc.vector.reduce_sum(
                    pb_all[:, col : col + 1], escrb, axis=mybir.AxisListType.X
                )

            def kv_knp(j16):
                knp = ps.tile([P, DIM], f32, tag="u", bufs=1, name=f"knp{j16}")
                for d in range(4):
                    nc.tensor.matmul(
                        knp,
                        xT[d][:, P * j16 : P * (j16 + 1)],
                        wqb[d][:, INNER : 2 * INNER],
                        start=(d == 0), stop=(d == 3),
                    )
                kn_sb = sb.tile([P, DIM], bf16, tag="kn", bufs=2, name=f"kn{j16}")
                nc.vector.tensor_copy(kn_sb, knp)
                return kn_sb

            def kv_vnp(j16):
                vnp = ps.tile([P, DIM], f32, tag="u", bufs=1, name=f"vnp{j16}")
                for d in range(4):
                    nc.tensor.matmul(
                        vnp,
                        xT[d][:, P * j16 : P * (j16 + 1)],
                        wqb[d][:, 2 * INNER : 3 * INNER],
                        start=(d == 0), stop=(d == 3),
                    )
                vn_sb = sb.tile([P, DIM], bf16, tag="vn", bufs=2, name=f"vn{j16}")
                nc.vector.tensor_copy(vn_sb, vnp)
                return vn_sb

            def kv_ktv(j16, kn_sb, vn_sb):
                kvp = ps.tile([P, 2 * P], f32, tag="u", bufs=1, name=f"kvp{j16}")
                for h in range(HEADS):
                    nc.tensor.matmul(
                        kvp[(h % 2) * DH : (h % 2 + 1) * DH, DH * (h // 2) : DH * (h // 2 + 1)],
                        vn_sb[:, DH * h : DH * (h + 1)],
                        kn_sb[:, DH * h : DH * (h + 1)],
                        start=True, stop=True,
                    )
                cvp = ps.tile([1, INNER], f32, tag="u", bufs=1, name=f"cvp{j16}")
                nc.tensor.matmul(cvp, ones_col, vn_sb, start=True, stop=True)
                if j16 == 0:
                    nc.vector.tensor_copy(ktvT_acc, kvp[:, 0 : 2 * P])
                    nc.vector.tensor_copy(colv_acc, cvp)
                else:
                    nc.vector.tensor_add(ktvT_acc, ktvT_acc, kvp[:, 0 : 2 * P])
                    nc.vector.tensor_add(colv_acc, colv_acc, cvp)

            def finalize_ktv():
                nc.vector.tensor_scalar_mul(ktvT_sb, ktvT_acc, SCALE)
                for t in range(4):
                    nc.gpsimd.memset(bd[t], 0.0)
                    nc.vector.tensor_copy(bd[t][0:DH, 0:DH], ktvT_sb[0:DH, DH * t : DH * (t + 1)])
                    nc.vector.tensor_copy(bd[t][DH:P, DH:P], ktvT_sb[DH:P, DH * t : DH * (t + 1)])
                for t in range(4):
                    ap_ = ps.tile([P, DIM], f32, tag="u", bufs=1, name=f"ap{t}")
                    nc.tensor.matmul(ap_, bd[t], wo[t], start=True, stop=True)
                    nc.vector.tensor_copy(At_sb[t], ap_)
                for t in range(4):
                    bp = ps.tile([P, P], f32, tag="u", bufs=1, name=f"bp{t}")
                    nc.tensor.transpose(bp, bd[t], identf)
                    nc.vector.tensor_copy(B_sb[t], bp)
                cvt = ps.tile([P, 4], f32, tag="u", bufs=1, name="cvt")
                for t in range(4):
                    nc.tensor.matmul(
                        cvt[:, t : t + 1],
                        colv_acc[:, P * t : P * (t + 1)],
                        identf[0:1, 0:1],
                        start=True, stop=True,
                    )
                nc.vector.tensor_copy(colvT_sb, cvt)
                for t in range(4):
                    nc.vector.tensor_scalar_mul(mkA[t], maskA[t], colvT_sb[:, t : t + 1])
                cp = ps.tile([8, DIM], f32, tag="u", bufs=1, name="cp")
                for t in range(4):
                    nc.tensor.matmul(cp, mkA[t], wo[t], start=(t == 0), stop=(t == 3))
                nc.vector.tensor_scalar_mul(CCx, cp, -1.0)
                bc = ps.tile([8, INNER], f32, tag="u", bufs=1, name="bc")
                nc.tensor.matmul(bc, ones8, colv_acc, start=True, stop=True)
                nc.vector.tensor_mul(CCq, bc, maskB)

            def ln_batch(b):
                # fold partials + Ln for i-tiles [4b, 4b+4) -> cols 64b..64b+64
                c0, c1 = 64 * b, 64 * (b + 1)
                nc.vector.tensor_add(
                    se_all[:, c0:c1], pa_all[:, c0:c1], pb_all[:, c0:c1]
                )
                nc.scalar.activation(lse_all[:, c0:c1], se_all[:, c0:c1], AF.Ln)

            def assemble(it, tags=("u",)):
                def atile(width, name):
                    tag = tags[atile.i % len(tags)]
                    atile.i += 1
                    return ps.tile(
                        [P, width], f32, tag=tag,
                        bufs=(1 if tag in ("pb", "u") else None), name=name
                    )
                atile.i = 0
                ltx = atile(P, f"ltx{it}")[0:8, :]
                nc.tensor.transpose(ltx, lse_all[:, 16 * it : 16 * it + 8], identf)
                ltq = atile(P, f"ltq{it}")[0:8, :]
                nc.tensor.transpose(ltq, lse_all[:, 16 * it + 8 : 16 * it + 16], identf)
                lx_sb = sb.tile([8, P], f32, tag="lx", bufs=2, name=f"lx{it}")
                lq_sb = sb.tile([8, P], f32, tag="lq", bufs=2, name=f"lq{it}")
                nc.vector.tensor_copy(lx_sb, ltx)
                nc.vector.tensor_copy(lq_sb, ltq)

                xp = atile(DIM, f"xp{it}")
                for t in range(4):
                    nc.tensor.matmul(
                        xp, QT[t][:, P * it : P * (it + 1)], At_sb[t],
                        start=(t == 0), stop=False,
                    )
                nc.tensor.matmul(xp, lx_sb, CCx, start=False, stop=True)
                xst = sb.tile([P, DIM], f32, tag="xst", bufs=2, name=f"xst{it}")
                nc.vector.tensor_add(xst, xp, xn[it])
                nc.sync.dma_start(xnew[P * it : P * (it + 1), :], xst)

                qp = atile(INNER, f"qpo{it}")
                for t in range(4):
                    reg = qp[:, P * t : P * (t + 1)]
                    nc.tensor.matmul(
                        reg, lq_sb, CCq[:, P * t : P * (t + 1)], start=True, stop=False
                    )
                    nc.tensor.matmul(
                        reg, q2T[t][:, P * it : P * (it + 1)], B_sb[t],
                        start=False, stop=True,
                    )
                qst = sb.tile([P, INNER], f32, tag="qst", bufs=2, name=f"qst{it}")
                nc.vector.tensor_add(qst, qp, q2n[it])
                nc.sync.dma_start(qnew[P * it : P * (it + 1), :], qst)

            # ---------------- prologue: finely interleaved ----------------
            for j in range(4):
                nc.sync.dma_start(xn[j], x_b[P * j : P * (j + 1), :])
                nc.vector.tensor_copy(xnb[j], xn[j])
            for d in range(4):
                for c0 in (INNER, 0, 2 * INNER):  # K cols first, then Q, then V
                    nc.sync.dma_start(
                        wq[d][:, c0 : c0 + INNER],
                        w_qkv[P * d : P * (d + 1), c0 : c0 + INNER],
                    )
                    nc.vector.tensor_copy(
                        wqb[d][:, c0 : c0 + INNER], wq[d][:, c0 : c0 + INNER]
                    )
            for g in range(4):
                for k in range(4):
                    j = 4 * g + k
                    if j >= 4:
                        nc.sync.dma_start(xn[j], x_b[P * j : P * (j + 1), :])
                        nc.vector.tensor_copy(xnb[j], xn[j])
                for d in range(4):
                    transpose_group(xT[d], xnb, d, g, "tx")
                # KT0 chunk g ready as soon as its xT columns exist
                project_chunk(KT[0], INNER, g, f"kp0{g}")
                if g < 2:
                    project_chunk(QT[0], 0, g, f"qp0{g}")
            for d in range(4):
                nc.sync.dma_start(wo[d], w_out[P * d : P * (d + 1), :])
            for j in range(NIT):
                nc.sync.dma_start(q2n[j], qoir_r[P * j : P * (j + 1), :])
                nc.vector.tensor_copy(q2nb[j], q2n[j])
            for g in range(2):
                for d in range(4):
                    transpose_group(q2T[d], q2nb, d, g, "tq")

            # late projections: backfill
            with backfill():
                for t in range(1, 4):
                    for jc in range(4):
                        project_chunk(KT[t], INNER + P * t, jc, f"kp{t}{jc}", psname="u")
                    for ic in range(2):
                        project_chunk(QT[t], P * t, ic, f"qq{t}{ic}", psname="u")

            # ---------------- dots stream with utility backfill ----------------
            kn_live = {}
            vn_live = {}
            for t in range(4):
                for it in range(NIT):
                    for s in range(2):
                        for h in (2 * t, 2 * t + 1):
                            dots_pair(it, s, h)
                    slot = 8 * t + it
                    if slot < 16:
                        with backfill():
                            kn = kv_knp(slot)
                            vn = kv_vnp(slot)
                            kv_ktv(slot, kn, vn)
                    if slot == 17:
                        with backfill():
                            finalize_ktv()
                    if t == 3 and it == 4:
                        ln_batch(0)
                        with backfill():
                            for a_it in range(4):
                                assemble(a_it)

            # ---------------- tail ----------------
            ln_batch(1)
            for a_it in range(4, NIT):
                assemble(a_it, tags=("u", "pb", "dots", "dots"))

    nc.compile()
    return nc


_CACHE = {}


def _get_nc():
    if "nc" not in _CACHE:
        _CACHE["nc"] = build_bass()
    return _CACHE["nc"]


def _shard_inputs(x, qoir):
    """Per-core input maps. Core c: batch c//2, row-half c%2, own rows first."""
    in_maps = []
    for c in range(NCORES):
        b, half = c // 2, c % 2
        mine = x[b, half * ROWS : (half + 1) * ROWS]
        other = x[b, (1 - half) * ROWS : (2 - half) * ROWS]
        in_maps.append(
            {
                "x_b": np.ascontiguousarray(np.concatenate([mine, other], axis=0)),
                "qoir_r": np.ascontiguousarray(qoir[b, half * ROWS : (half + 1) * ROWS]),
            }
        )
    return in_maps


def _ident():
    return np.eye(P, dtype=np.float32)


def _maskB():
    mb = np.zeros((8, INNER), dtype=np.float32)
    for h in range(8):
        mb[h, DH * h : DH * (h + 1)] = -1.0
    return mb


def kernel(x, qoir, w_qkv, w_out):
    from concourse.bass_utils import run_bass_kernel_spmd

    x = np.asarray(x, dtype=np.float32)
    qoir = np.asarray(qoir, dtype=np.float32)
    w_qkv = np.ascontiguousarray(np.asarray(w_qkv, dtype=np.float32))
    w_out = np.ascontiguousarray(np.asarray(w_out, dtype=np.float32))

    nc = _get_nc()
    in_maps = _shard_inputs(x, qoir)
    for m in in_maps:
        m["w_qkv"] = w_qkv
        m["w_out"] = w_out
        m["maskB_in"] = _maskB()
        m["ident_in"] = _ident()

    res = run_bass_kernel_spmd(nc, in_maps, core_ids=list(range(NCORES)))
    x_new = np.empty((B, N, DIM), dtype=np.float32)
    q_new = np.empty((B, N, INNER), dtype=np.float32)
    for c in range(NCORES):
        b, half = c // 2, c % 2
        rows = slice(half * ROWS, (half + 1) * ROWS)
        x_new[b, rows] = res.results[c]["xnew_p"]
        q_new[b, rows] = res.results[c]["qnew_p"]
    return (x_new, q_new)
